# revision 16
# baseline (speedup 1.0000x reference)
"""Trainium2 Bass kernel for nn_BiLSTM_18184891531327.

word-emb + grouped char-CNN -> 2-layer BiLSTM -> MHA self-attention -> CRF
(viterbi decode + NLL loss + head-averaged attention weights).

Sharding: 8 identical SPMD programs, each core owns 4 of the 32 batch
elements and computes both LSTM directions locally (the backward chain simply
indexes time steps in descending order inside the fully-unrolled loop).

Numerics tricks:
- sigmoid(x) = (tanh(x/2)+1)/2: i/f/o rows of every W/b are pre-scaled by 0.5
  on the host so one tanh activation covers all 4 gates; gate order is
  permuted to [i,f,o,g].
- h is stored as 2*h; every weight consuming h is pre-scaled by 0.5.
- CRF forward (logsumexp) runs in the exp domain as a [20,20] matmul chain
  with periodic renormalization; logs of the banked normalizers are summed at
  the end.
- Viterbi history (argmaxes) is recomputed in one batched pass from the
  stored score history; backtracking runs on the host (pointer-chasing over
  8k ints).
"""

import numpy as np

import concourse.bass as bass
import concourse.mybir as mybir
import concourse.tile as tile
from concourse.vector_clock import ScopedClock
from concourse.bass_utils import run_bass_kernel_spmd
from concourse.masks import make_identity

FP = mybir.dt.float32
AX = mybir.AxisListType
OP = mybir.AluOpType
AF = mybir.ActivationFunctionType

V, EW, EC, NF, KW = 50001, 300, 32, 4, 3
FOUT = EC * NF          # 128
H = 256
T = 20
S, B, W = 256, 32, 16
NH, E2 = 8, 512
DH = E2 // NH           # 64
NC = 8
BL = B // NC            # 4
NTOK = S * BL           # 1024, token t = s*BL + b
NW = NTOK * W
EWP = 320               # padded word-emb row
G4 = 4 * H              # 1024
NBLK = G4 // 128        # 8
PBJ = T * BL            # 80 partitions (j-major: p = j*BL + b)
BIG = 1.0e6
RENORM_EVERY = 6

# ---------------------------------------------------------------------------
# tile drain patch: walrus here accepts only 1 sync-wait per CTRL inst.
# ---------------------------------------------------------------------------


_WSPLIT_N = [0]


def _split_multiwaits(nc):
    """walrus here allows a single sync-wait per instruction: move extra
    waits onto same-engine NoOps inserted right before the instruction."""
    for fn in nc.m.functions:
        for bb in fn.blocks:
            insts = list(bb.instructions)
            if not any(
                i.sync_info is not None
                and i.sync_info.on_wait is not None
                and len(i.sync_info.on_wait) > 1
                for i in insts
            ):
                continue
            out = []
            for inst in insts:
                si = inst.sync_info
                if (
                    si is not None
                    and si.on_wait is not None
                    and len(si.on_wait) > 1
                ):
                    waits = list(si.on_wait)
                    si.on_wait = waits[-1:]
                    for w in waits[:-1]:
                        _WSPLIT_N[0] += 1
                        nop = mybir.InstNoOp(
                            name="WSPLIT-%d" % _WSPLIT_N[0], ins=[], outs=[]
                        )
                        nop.engine = inst.engine
                        nop.sync_info = mybir.SyncInfo(
                            on_wait=[w], on_update=[]
                        )
                        nc.register_instruction(nop, overwrite=True)
                        out.append(nop)
                out.append(inst)
            bb.instructions = out


def _patched_drain_and_barrier(self, tick_clock, wait_clock):
    drain_inst = self.nc.sync.drain()
    wait_clock.add_sem_waits(
        drain_inst.ins, ScopedClock({None: tick_clock.global_clock})
    )
    si = drain_inst.ins.sync_info
    if si is not None and si.on_wait is not None and len(si.on_wait) > 1:
        waits = list(si.on_wait)
        si.on_wait = waits[:1]
        for w in waits[1:]:
            d2 = self.nc.sync.drain()
            si2 = d2.ins.sync_info
            if si2 is None:
                d2.ins.sync_info = mybir.SyncInfo(on_wait=[w], on_update=[])
            else:
                si2.on_wait = [w]
    self.nc.all_engine_barrier()
    popped = self.nc._tile_sem_poison_stack.pop()
    assert popped is self._sem_poison
    self.nc.clear_and_free_semaphores(list(self.sems.allocated().values()))
    self.nc.all_engine_barrier()


tile.TileContext._drain_and_barrier = _patched_drain_and_barrier


# ---------------------------------------------------------------------------
# host-side prep
# ---------------------------------------------------------------------------


def _gate_perm_scale(a):
    """torch gate rows [i,f,g,o] -> [i,f,o,g]; i/f/o scaled by 0.5."""
    a = np.asarray(a, np.float32)
    i, f, g, o = a[0:H], a[H : 2 * H], a[2 * H : 3 * H], a[3 * H : 4 * H]
    return np.concatenate([0.5 * i, 0.5 * f, 0.5 * o, g], axis=0)


def _chunkT(wT, chunks):
    out = np.zeros((len(chunks), 128, wT.shape[1]), np.float32)
    for ci, (s0, n) in enumerate(chunks):
        out[ci, :n] = wT[s0 : s0 + n]
    return np.ascontiguousarray(out)


def _bias_tiles(b):
    return np.ascontiguousarray(np.asarray(b, np.float32).reshape(NBLK, 128).T)


def prep_shared(inp):
    d = {}
    we = np.asarray(inp["word_emb"], np.float32)
    wep = np.zeros((V, EWP), np.float32)
    wep[:, :EW] = we
    d["word_emb_pad"] = wep

    tab = np.asarray(inp["char_emb_tab"], np.float32)   # [128, 32]
    cw = np.asarray(inp["conv_w"], np.float32)          # [128, 1, 3]
    fw = np.zeros((KW, 128, FOUT), np.float32)
    for k in range(KW):
        fw[k] = tab[:, np.arange(FOUT) // NF] * cw[None, :, 0, k]
    d["fw_conv"] = fw
    d["conv_b"] = np.asarray(inp["conv_b"], np.float32).reshape(FOUT, 1)

    l0_chunks = [(0, 128), (128, 128), (256, 44), (300, 128)]
    l1_chunks = [(0, 128), (128, 128), (256, 128), (384, 128)]
    for layer, chunks in ((0, l0_chunks), (1, l1_chunks)):
        for dr in ("f", "b"):
            sfx = "l%d%s" % (layer, dr)
            wih = _gate_perm_scale(inp["w_ih_" + sfx])
            whh = _gate_perm_scale(inp["w_hh_" + sfx]) * 0.5
            if layer == 1:
                wih = wih * 0.5
            bb = _gate_perm_scale(
                (
                    np.asarray(inp["b_ih_" + sfx], np.float32)
                    + np.asarray(inp["b_hh_" + sfx], np.float32)
                )[:, None]
            )[:, 0]
            d["wih_" + sfx] = _chunkT(np.ascontiguousarray(wih.T), chunks)
            d["whh_" + sfx] = _chunkT(
                np.ascontiguousarray(whh.T), [(0, 128), (128, 128)]
            )
            d["bias_" + sfx] = _bias_tiles(bb)

    ipw = np.asarray(inp["in_proj_w"], np.float32)
    ipb = np.asarray(inp["in_proj_b"], np.float32)
    sc = 1.0 / np.sqrt(np.float32(DH))
    ch4 = [(0, 128), (128, 128), (256, 128), (384, 128)]
    d["wqT"] = _chunkT(np.ascontiguousarray((ipw[0:E2] * 0.5 * sc).T), ch4)
    d["wkT"] = _chunkT(np.ascontiguousarray((ipw[E2 : 2 * E2] * 0.5).T), ch4)
    d["wvT"] = _chunkT(np.ascontiguousarray((ipw[2 * E2 :] * 0.5).T), ch4)
    d["bq"] = np.ascontiguousarray((ipb[0:E2] * sc).reshape(4, 128).T)
    d["bk"] = np.ascontiguousarray(ipb[E2 : 2 * E2].reshape(4, 128).T)
    d["bv"] = np.ascontiguousarray(ipb[2 * E2 :].reshape(1, E2))
    opwT = np.ascontiguousarray(np.asarray(inp["out_proj_w"], np.float32).T)
    d["woT"] = np.ascontiguousarray(opwT.reshape(NH, DH, E2))
    d["bo"] = np.ascontiguousarray(
        np.asarray(inp["out_proj_b"], np.float32).reshape(4, 128).T
    )
    d["fcT"] = _chunkT(
        np.ascontiguousarray(np.asarray(inp["fc_w"], np.float32).T), ch4
    )
    d["bfc"] = np.asarray(inp["fc_b"], np.float32).reshape(T, 1)

    trans = np.asarray(inp["trans"], np.float32)
    start = np.asarray(inp["start_trans"], np.float32)
    end = np.asarray(inp["end_trans"], np.float32)
    p_j = np.repeat(np.arange(T), BL)              # p=(j,b): j index
    d["transT_rep"] = np.ascontiguousarray(trans.T[p_j])          # [80,20]
    d["P_const"] = (p_j[:, None] == np.arange(T)[None, :]).astype(np.float32)
    bcol = np.tile(np.arange(BL), T)
    d["Bsel"] = (bcol[:, None] == bcol[None, :]).astype(np.float32)
    d["expTrans"] = np.exp(trans).astype(np.float32)
    d["expEnd"] = np.exp(end).astype(np.float32).reshape(T, 1)
    d["endT_rep"] = np.ascontiguousarray(end[p_j].reshape(PBJ, 1))
    d["startT_rep"] = np.ascontiguousarray(start[p_j].reshape(PBJ, 1))
    d["argC"] = np.ascontiguousarray(
        np.broadcast_to(np.arange(T, dtype=np.float32)[None, :] + BIG, (PBJ, T))
    )
    d["transT"] = np.ascontiguousarray(trans.T)
    d["start_col"] = start.reshape(T, 1).astype(np.float32)
    d["end_col"] = end.reshape(T, 1).astype(np.float32)
    return d


def prep_core(inp, core):
    d = {}
    b0 = core * BL
    words = np.asarray(inp["words"]).astype(np.int64)[:, b0 : b0 + BL]
    chars = np.asarray(inp["chars"]).astype(np.int64)[b0 : b0 + BL]
    tags = np.asarray(inp["tags"]).astype(np.int64)[:, b0 : b0 + BL]
    widx = words.reshape(NTOK).astype(np.int32)
    d["widx"] = np.ascontiguousarray(widx.reshape(NTOK // 128, 128))
    d["chars_f"] = np.ascontiguousarray(
        chars.transpose(1, 0, 2).reshape(1, NW).astype(np.float32)
    )
    d["tags_f"] = np.ascontiguousarray(
        tags.reshape(1, NTOK).astype(np.float32)
    )
    return d


# ---------------------------------------------------------------------------
# device program
# ---------------------------------------------------------------------------

SLS = S  # lstm/crf step count (shrink for debugging only)


def build_program(debug=False):
    nc = bass.Bass("TRN2")

    def par(name, shape, dtype=FP):
        return nc.declare_dram_parameter(name, list(shape), dtype, isOutput=False)

    def out_par(name, shape, dtype=FP):
        return nc.declare_dram_parameter(name, list(shape), dtype, isOutput=True)

    io = {}
    io["word_emb_pad"] = par("word_emb_pad", [V, EWP])
    io["fw_conv"] = par("fw_conv", [KW, 128, FOUT])
    io["conv_b"] = par("conv_b", [FOUT, 1])
    for layer in (0, 1):
        for dr in ("f", "b"):
            sfx = "l%d%s" % (layer, dr)
            io["wih_" + sfx] = par("wih_" + sfx, [4, 128, G4])
            io["whh_" + sfx] = par("whh_" + sfx, [2, 128, G4])
            io["bias_" + sfx] = par("bias_" + sfx, [128, NBLK])
    for nm, shp in (
        ("wqT", [4, 128, E2]),
        ("wkT", [4, 128, E2]),
        ("wvT", [4, 128, E2]),
        ("bq", [128, 4]),
        ("bk", [128, 4]),
        ("bv", [1, E2]),
        ("woT", [NH, DH, E2]),
        ("bo", [128, 4]),
        ("fcT", [4, 128, T]),
        ("bfc", [T, 1]),
        ("transT_rep", [PBJ, T]),
        ("P_const", [PBJ, T]),
        ("Bsel", [PBJ, PBJ]),
        ("expTrans", [T, T]),
        ("expEnd", [T, 1]),
        ("endT_rep", [PBJ, 1]),
        ("startT_rep", [PBJ, 1]),
        ("argC", [PBJ, T]),
        ("transT", [T, T]),
        ("start_col", [T, 1]),
        ("end_col", [T, 1]),
        ("chars_f", [1, NW]),
        ("tags_f", [1, NTOK]),
    ):
        io[nm] = par(nm, shp)
    io["widx"] = par("widx", [NTOK // 128, 128], mybir.dt.int32)

    io["attnw"] = out_par("attnw", [BL, 128, 2, S])  # (b, kpart, khalf, q)
    io["histv"] = out_par("histv", [PBJ, S])   # cols 0..S-2 hist, col S-1 final score+end
    io["loss_part"] = out_par("loss_part", [1, 1])
    if debug:
        io["dbg_xT"] = out_par("dbg_xT", [4, 128, NTOK])
        io["dbg_gx"] = out_par("dbg_gx", [128, NBLK, NTOK])
        io["dbg_h01"] = out_par("dbg_h01", [2, 128, 4, NTOK])
        io["dbg_qkv"] = out_par("dbg_qkv", [3, 128, 4096])
        io["dbg_emis"] = out_par("dbg_emis", [T, NTOK])
        io["dbg_score"] = out_par("dbg_score", [PBJ, S])
        io["dbg_den"] = out_par("dbg_den", [1, BL])
        io["dbg_bank"] = out_par("dbg_bank", [1, (S - 2) // RENORM_EVERY + 2, BL])
        io["dbg_num"] = out_par("dbg_num", [1, BL])

    io["emis_d"] = nc.dram_tensor("emis_d", [T, BL, S], FP)

    with tile.TileContext(nc) as tc:
        _body(nc, tc, io, debug)
    _split_multiwaits(nc)
    return nc


def _body(nc, tc, io, debug):
    MI32 = mybir.dt.int32

    with (
        tc.tile_pool(name="const", bufs=1) as constp,
        tc.tile_pool(name="state", bufs=1) as statep,
    ):
        ident = constp.tile([128, 128], FP)
        make_identity(nc, ident[:])
        iota_i = constp.tile([128, 1], MI32)
        nc.gpsimd.iota(iota_i[:], pattern=[[0, 1]], base=0, channel_multiplier=1)
        iotaf = constp.tile([128, 1], FP)
        nc.vector.tensor_copy(out=iotaf[:], in_=iota_i[:])
        ones_row = constp.tile([1, 128], FP)
        nc.vector.memset(ones_row[:], 1.0)
        ones_col128 = constp.tile([128, 1], FP)
        nc.vector.memset(ones_col128[:], 1.0)


        # ==================================================================
        # Phase A: embeddings + char conv -> xT [128, 4, NTOK]
        # (chunks: wd 0:128, wd 128:256, wd 256:300 (44 rows), char 128)
        # ==================================================================
        with (
            tc.tile_pool(name="xT", bufs=1) as xTp,
            tc.tile_pool(name="wtok", bufs=3) as wtokp,
            tc.tile_pool(name="oh", bufs=1) as ohp,
            tc.tile_pool(name="wih", bufs=1) as wihp,
            tc.tile_pool(name="gx", bufs=1) as gxp,
            tc.tile_pool(name="misc_a", bufs=2) as miscp,
            tc.tile_pool(name="ps_a", bufs=2, space="PSUM") as psA,
            tc.tile_pool(name="ps_g", bufs=2, space="PSUM") as psG,
        ):
            xT = xTp.tile([128, 4, NTOK], FP)
            nc.vector.memset(xT[:, 2, :], 0.0)

            widx_sb = constp.tile([128, NTOK // 128], MI32)
            nc.sync.dma_start(
                out=widx_sb[:], in_=io["widx"][:, :].rearrange("j p -> p j")
            )
            for j in range(NTOK // 128):
                wtok = wtokp.tile([128, EWP], FP, tag="wtok")
                nc.gpsimd.indirect_dma_start(
                    out=wtok[:],
                    out_offset=None,
                    in_=io["word_emb_pad"][:, :],
                    in_offset=bass.IndirectOffsetOnAxis(
                        ap=widx_sb[:, j : j + 1], axis=0
                    ),
                )
                for ci, (c0, n) in enumerate([(0, 128), (128, 128), (256, 44)]):
                    pt = psA.tile([128, 512], FP, tag="bc", name="tp")[:, :128]
                    nc.tensor.transpose(
                        out=pt[:n, :], in_=wtok[:, c0 : c0 + n], identity=ident[:]
                    )
                    nc.vector.tensor_copy(
                        out=xT[:n, ci, j * 128 : (j + 1) * 128], in_=pt[:n, :]
                    )

            # char one-hot + fused conv + maxpool -> xT chunk 3
            fw_sb = constp.tile([128, KW, FOUT], FP)
            nc.sync.dma_start(
                out=fw_sb[:], in_=io["fw_conv"][:, :, :].rearrange("k p f -> p k f")
            )
            convb_sb = constp.tile([128, 1], FP)
            nc.sync.dma_start(out=convb_sb[:], in_=io["conv_b"][:, :])

            SEG = 128   # tokens per one-hot segment
            CH = 32     # tokens per conv chunk
            PW = W - KW + 1  # 14
            for seg in range(NTOK // SEG):
                chseg = miscp.tile([1, SEG * W], FP, tag="chseg", bufs=1)
                nc.sync.dma_start(
                    out=chseg[:],
                    in_=io["chars_f"][:, seg * SEG * W : (seg + 1) * SEG * W],
                )
                oh = ohp.tile([128, SEG, W], FP, tag="oh")
                for q in range(SEG * W // 512):
                    pb = psA.tile([128, 512], FP, tag="bc")
                    nc.tensor.matmul(
                        out=pb[:],
                        lhsT=ones_row[:],
                        rhs=chseg[:, q * 512 : (q + 1) * 512],
                        start=True,
                        stop=True,
                    )
                    nc.vector.tensor_tensor(
                        out=oh[:].rearrange("p a b -> p (a b)")[
                            :, q * 512 : (q + 1) * 512
                        ],
                        in0=pb[:],
                        in1=iotaf[:].to_broadcast([128, 512]),
                        op=OP.is_equal,
                    )
                for c in range(SEG // CH):
                    pc = psA.tile([128, CH, PW], FP, tag="conv")
                    for k in range(KW):
                        nc.tensor.matmul(
                            out=pc[:],
                            lhsT=fw_sb[:, k, :],
                            rhs=oh[:, c * CH : (c + 1) * CH, k : k + PW],
                            start=(k == 0),
                            stop=(k == KW - 1),
                        )
                    mx = miscp.tile([128, CH], FP, tag="mx")
                    nc.vector.tensor_reduce(
                        out=mx[:], in_=pc[:], axis=AX.X, op=OP.max
                    )
                    nc.vector.tensor_scalar_add(
                        out=xT[:, 3, seg * SEG + c * CH :][:, :CH],
                        in0=mx[:],
                        scalar1=convb_sb[:],
                    )
            if debug:
                nc.sync.dma_start(
                    out=io["dbg_xT"][:, :, :],
                    in_=xT[:].rearrange("p c n -> c p n"),
                )

            # ==============================================================
            # gates_x + LSTM, both layers, both directions
            # ==============================================================
            h0_all = statep.tile([128, 4, NTOK], FP)  # chunks [f0,f1,b0,b1]
            whh_sb = {}
            bias_sb = {}
            for layer in (0, 1):
                for dr in ("f", "b"):
                    sfx = "l%d%s" % (layer, dr)
                    whh_sb[sfx] = wihp.tile(
                        [128, 2, G4], FP, tag="whh_" + sfx, name="whh_" + sfx
                    )
                    nc.sync.dma_start(
                        out=whh_sb[sfx][:],
                        in_=io["whh_" + sfx][:, :, :].rearrange("k p g -> p k g"),
                    )
                    bias_sb[sfx] = constp.tile(
                        [128, NBLK], FP, tag="bias_" + sfx, name="bias_" + sfx
                    )
                    nc.sync.dma_start(
                        out=bias_sb[sfx][:], in_=io["bias_" + sfx][:, :]
                    )

            gx = {
                "f": gxp.tile([128, NBLK, NTOK], FP, tag="gxf", name="gxf"),
                "b": gxp.tile([128, NBLK, NTOK], FP, tag="gxb", name="gxb"),
            }

            def gates_x(layer, dr, rhs_chunks):
                sfx = "l%d%s" % (layer, dr)
                wt = wihp.tile([128, 4, G4], FP, tag="wih")
                nc.sync.dma_start(
                    out=wt[:],
                    in_=io["wih_" + sfx][:, :, :].rearrange("k p g -> p k g"),
                )
                for m in range(NBLK):
                    for nch in range(NTOK // 512):
                        ps = psG.tile([128, 512], FP, tag="g")
                        for kc in range(4):
                            nc.tensor.matmul(
                                out=ps[:],
                                lhsT=wt[:, kc, m * 128 : (m + 1) * 128],
                                rhs=rhs_chunks(kc, nch),
                                start=(kc == 0),
                                stop=(kc == 3),
                            )
                        nc.vector.tensor_scalar_add(
                            out=gx[dr][:, m, nch * 512 :][:, :512],
                            in0=ps[:],
                            scalar1=bias_sb[sfx][:, m : m + 1],
                        )

            def lstm_chain(layer, dr, h_dst):
                """h_dst: [128, 2, NTOK] view of h{layer}_all chunks."""
                sfx = "l%d%s" % (layer, dr)
                c_t = statep.tile([128, 2, BL], FP, tag="c_" + sfx)
                nc.vector.memset(c_t[:], 0.0)
                for u in range(SLS):
                    s = u if dr == "f" else SLS - 1 - u
                    gxs = gx[dr][:, :, s * BL : (s + 1) * BL]   # [128,8,BL]
                    if u == 0:
                        gsrc = gxs
                    else:
                        sp = s + 1 if dr == "b" else s - 1
                        hprev = h_dst[:, :, sp * BL : (sp + 1) * BL]
                        ps = psG.tile([128, 512], FP, tag="g", name="lps")[:, : NBLK * BL]
                        for m in range(NBLK):
                            for kc in range(2):
                                nc.tensor.matmul(
                                    out=ps[:, m * BL : (m + 1) * BL],
                                    lhsT=whh_sb[sfx][:, kc, m * 128 :][:, :128],
                                    rhs=hprev[:, kc, :],
                                    start=(kc == 0),
                                    stop=(kc == 1),
                                )
                        gfull = miscp.tile([128, NBLK, BL], FP, tag="gf" + dr)
                        nc.vector.tensor_tensor(
                            out=gfull[:], in0=ps[:].rearrange(
                                "p (m b) -> p m b", m=NBLK
                            ), in1=gxs, op=OP.add
                        )
                        gsrc = gfull[:]
                    tall = miscp.tile([128, NBLK, BL], FP, tag="ta" + dr)
                    nc.scalar.activation(out=tall[:], in_=gsrc, func=AF.Tanh)
                    ti = tall[:, 0:2, :]
                    tf = tall[:, 2:4, :]
                    to = tall[:, 4:6, :]
                    tg = tall[:, 6:8, :]
                    u1 = miscp.tile([128, 2, BL], FP, tag="u1" + dr)
                    nc.vector.scalar_tensor_tensor(
                        out=u1[:], in0=tf, scalar=1.0, in1=c_t[:],
                        op0=OP.add, op1=OP.mult,
                    )
                    v1 = miscp.tile([128, 2, BL], FP, tag="v1" + dr)
                    nc.vector.scalar_tensor_tensor(
                        out=v1[:], in0=ti, scalar=1.0, in1=tg,
                        op0=OP.add, op1=OP.mult,
                    )
                    x2 = miscp.tile([128, 2, BL], FP, tag="x2" + dr)
                    nc.vector.tensor_tensor(
                        out=x2[:], in0=u1[:], in1=v1[:], op=OP.add
                    )
                    tc2 = miscp.tile([128, 2, BL], FP, tag="tc" + dr)
                    nc.scalar.activation(
                        out=tc2[:], in_=x2[:], func=AF.Tanh, scale=0.5
                    )
                    nc.vector.tensor_scalar_mul(
                        out=c_t[:], in0=x2[:], scalar1=0.5
                    )
                    nc.vector.scalar_tensor_tensor(
                        out=h_dst[:, :, s * BL : (s + 1) * BL],
                        in0=to, scalar=1.0, in1=tc2[:],
                        op0=OP.add, op1=OP.mult,
                    )

            # layer 0
            gates_x(0, "f", lambda kc, nch: xT[:, kc, nch * 512 :][:, :512])
            gates_x(0, "b", lambda kc, nch: xT[:, kc, nch * 512 :][:, :512])
            if debug:
                nc.sync.dma_start(out=io["dbg_gx"][:, :, :], in_=gx["f"][:])
            lstm_chain(0, "f", h0_all[:, 0:2, :])
            lstm_chain(0, "b", h0_all[:, 2:4, :])
            # layer 1 (input = h0_all, stored doubled; wih_l1 pre-halved)
            h1_all = statep.tile([128, 4, NTOK], FP)
            gates_x(1, "f", lambda kc, nch: h0_all[:, kc, nch * 512 :][:, :512])
            gates_x(1, "b", lambda kc, nch: h0_all[:, kc, nch * 512 :][:, :512])
            lstm_chain(1, "f", h1_all[:, 0:2, :])
            lstm_chain(1, "b", h1_all[:, 2:4, :])
            if debug:
                nc.sync.dma_start(
                    out=io["dbg_h01"][0, :, :, :], in_=h0_all[:]
                )
                nc.sync.dma_start(
                    out=io["dbg_h01"][1, :, :, :], in_=h1_all[:]
                )

        # ==================================================================
        # Phase F: attention + fc -> emisT [20, NTOK]
        # ==================================================================
        with (
            tc.tile_pool(name="att", bufs=1) as attp,
            tc.tile_pool(name="attw", bufs=2) as attwp,
            tc.tile_pool(name="et", bufs=3) as etp,
            tc.tile_pool(name="ps_f", bufs=3, space="PSUM") as psF,
            tc.tile_pool(name="ps_f2", bufs=1, space="PSUM") as psF2,
        ):
            emisT = statep.tile([T, NTOK], FP)
            qT = attp.tile([128, 4, NTOK], FP)
            kT = attp.tile([128, 4, NTOK], FP)
            vv = attp.tile([128, NBLK, E2], FP)     # [s-half|b] token tiles
            av_all = attp.tile([DH, NH, S, BL], FP)
            attnT = attp.tile([128, 4, NTOK], FP)

            wq_sb = attwp.tile([128, 4, E2], FP, tag="wstream")
            nc.sync.dma_start(
                out=wq_sb[:], in_=io["wqT"][:, :, :].rearrange("k p g -> p k g")
            )
            wk_sb = attwp.tile([128, 4, E2], FP, tag="wstream")
            nc.sync.dma_start(
                out=wk_sb[:], in_=io["wkT"][:, :, :].rearrange("k p g -> p k g")
            )
            wv_sb = attwp.tile([128, 4, E2], FP, tag="wstream")
            nc.sync.dma_start(
                out=wv_sb[:], in_=io["wvT"][:, :, :].rearrange("k p g -> p k g")
            )
            bq_sb = constp.tile([128, 4], FP)
            nc.sync.dma_start(out=bq_sb[:], in_=io["bq"][:, :])
            bk_sb = constp.tile([128, 4], FP)
            nc.sync.dma_start(out=bk_sb[:], in_=io["bk"][:, :])
            bv_sb = constp.tile([128, E2], FP)
            nc.sync.dma_start(
                out=bv_sb[:], in_=io["bv"][:, :].to_broadcast([128, E2])
            )

            for dst, wsb, bsb in ((qT, wq_sb, bq_sb), (kT, wk_sb, bk_sb)):
                for m in range(4):
                    for nch in range(NTOK // 512):
                        ps = psF.tile([128, 512], FP, tag="f")
                        for kc in range(4):
                            nc.tensor.matmul(
                                out=ps[:],
                                lhsT=wsb[:, kc, m * 128 : (m + 1) * 128],
                                rhs=h1_all[:, kc, nch * 512 :][:, :512],
                                start=(kc == 0),
                                stop=(kc == 3),
                            )
                        nc.vector.tensor_scalar_add(
                            out=dst[:, m, nch * 512 :][:, :512],
                            in0=ps[:],
                            scalar1=bsb[:, m : m + 1],
                        )
            # v, token-major, tiles indexed (b, s-half)
            h1v = h1_all[:].rearrange("p c (s b) -> p c s b", b=BL)
            for b in range(BL):
                for kh in range(2):
                    ps = psF.tile([128, E2], FP, tag="f")
                    for kc in range(4):
                        nc.tensor.matmul(
                            out=ps[:],
                            lhsT=h1v[:, kc, kh * 128 : (kh + 1) * 128, b],
                            rhs=wv_sb[:, kc, :],
                            start=(kc == 0),
                            stop=(kc == 3),
                        )
                    nc.vector.tensor_tensor(
                        out=vv[:, b * 2 + kh, :], in0=ps[:], in1=bv_sb[:],
                        op=OP.add,
                    )
            if debug:
                nc.sync.dma_start(
                    out=io["dbg_qkv"][0], in_=qT[:].rearrange("p c n -> p (c n)")
                )
                nc.sync.dma_start(
                    out=io["dbg_qkv"][1], in_=kT[:].rearrange("p c n -> p (c n)")
                )
                nc.sync.dma_start(
                    out=io["dbg_qkv"][2], in_=vv[:].rearrange("p t e -> p (t e)")
                )

            qTv = qT[:].rearrange("p c (s b) -> p c s b", b=BL)
            kTv = kT[:].rearrange("p c (s b) -> p c s b", b=BL)
            for b in range(BL):
                awacc = attwp.tile([128, 2, S], FP, tag="awacc")
                for h in range(NH):
                    ch, off = h // 2, (h % 2) * DH
                    q_bh = qTv[off : off + DH, ch, :, b]    # [64, 256]
                    k_bh = kTv[off : off + DH, ch, :, b]
                    et = etp.tile([128, 2, S], FP, tag="et")
                    rs_ps = psF2.tile([1, S], FP, tag="rs")
                    for kh in range(2):
                        sc_ps = psF.tile([128, 512], FP, tag="f", name="scps")[:, :S]
                        nc.tensor.matmul(
                            out=sc_ps[:],
                            lhsT=k_bh[:, kh * 128 : (kh + 1) * 128],
                            rhs=q_bh,
                            start=True,
                            stop=True,
                        )
                        nc.scalar.activation(
                            out=et[:, kh, :], in_=sc_ps[:], func=AF.Exp
                        )
                    for kh in range(2):
                        nc.tensor.matmul(
                            out=rs_ps[:],
                            lhsT=ones_col128[:],
                            rhs=et[:, kh, :],
                            start=(kh == 0),
                            stop=(kh == 1),
                        )
                    rq = etp.tile([1, S], FP, tag="rq")
                    nc.vector.reciprocal(out=rq[:], in_=rs_ps[:])
                    # av_raw^T [64, 256] accumulated over k halves
                    av_ps = psF2.tile([DH, S], FP, tag="av")
                    for kh in range(2):
                        nc.tensor.matmul(
                            out=av_ps[:],
                            lhsT=vv[:, b * 2 + kh, h * DH : (h + 1) * DH],
                            rhs=et[:, kh, :],
                            start=(kh == 0),
                            stop=(kh == 1),
                        )
                    rqb_ps = psF2.tile([DH, S], FP, tag="rqb")
                    nc.tensor.matmul(
                        out=rqb_ps[:],
                        lhsT=ones_row[:, :DH],
                        rhs=rq[:],
                        start=True,
                        stop=True,
                    )
                    rqb = etp.tile([DH, S], FP, tag="rqbs")
                    nc.vector.tensor_copy(out=rqb[:], in_=rqb_ps[:])
                    nc.vector.tensor_tensor(
                        out=av_all[:, h, :, b], in0=av_ps[:], in1=rqb[:],
                        op=OP.mult,
                    )
                    # attn-weight accumulation: awacc (+)= et * rq_bc128
                    rqb128_ps = psF2.tile([128, S], FP, tag="rqb128")
                    nc.tensor.matmul(
                        out=rqb128_ps[:],
                        lhsT=ones_row[:],
                        rhs=rq[:],
                        start=True,
                        stop=True,
                    )
                    anorm = etp.tile([128, 2, S], FP, tag="anorm")
                    for kh in range(2):
                        nc.vector.tensor_tensor(
                            out=anorm[:, kh, :], in0=rqb128_ps[:],
                            in1=et[:, kh, :], op=OP.mult,
                        )
                    if h == 0:
                        nc.vector.tensor_scalar_mul(
                            out=awacc[:], in0=anorm[:], scalar1=1.0 / NH
                        )
                    else:
                        nc.vector.scalar_tensor_tensor(
                            out=awacc[:], in0=anorm[:], scalar=1.0 / NH,
                            in1=awacc[:], op0=OP.mult, op1=OP.add,
                        )
                nc.sync.dma_start(out=io["attnw"][b], in_=awacc[:])

            # out_proj + fc
            wo_sb = attwp.tile([DH, NH, E2], FP, tag="wstream")
            nc.sync.dma_start(
                out=wo_sb[:], in_=io["woT"][:, :, :].rearrange("h p g -> p h g")
            )
            bo_sb = constp.tile([128, 4], FP)
            nc.sync.dma_start(out=bo_sb[:], in_=io["bo"][:, :])
            for mo in range(4):
                for nch in range(NTOK // 512):
                    ps = psF.tile([128, 512], FP, tag="f")
                    for h in range(NH):
                        nc.tensor.matmul(
                            out=ps[:],
                            lhsT=wo_sb[:, h, mo * 128 : (mo + 1) * 128],
                            rhs=av_all[:, h, nch * 128 : (nch + 1) * 128, :],
                            start=(h == 0),
                            stop=(h == NH - 1),
                        )
                    nc.vector.tensor_scalar_add(
                        out=attnT[:, mo, nch * 512 :][:, :512],
                        in0=ps[:],
                        scalar1=bo_sb[:, mo : mo + 1],
                    )
            fc_sb = attwp.tile([128, 4, T], FP, tag="fcw")
            nc.sync.dma_start(
                out=fc_sb[:], in_=io["fcT"][:, :, :].rearrange("k p t -> p k t")
            )
            bfc_sb = constp.tile([T, 1], FP)
            nc.sync.dma_start(out=bfc_sb[:], in_=io["bfc"][:, :])
            for nch in range(NTOK // 512):
                ps = psF.tile([128, 512], FP, tag="f", name="emps")[:T, :]
                for kc in range(4):
                    nc.tensor.matmul(
                        out=ps[:],
                        lhsT=fc_sb[:, kc, :],
                        rhs=attnT[:, kc, nch * 512 :][:, :512],
                        start=(kc == 0),
                        stop=(kc == 3),
                    )
                nc.vector.tensor_scalar_add(
                    out=emisT[:, nch * 512 : (nch + 1) * 512],
                    in0=ps[:],
                    scalar1=bfc_sb[:],
                )
            if debug:
                nc.sync.dma_start(out=io["dbg_emis"][:, :], in_=emisT[:])

        # ==================================================================
        # Phase G: CRF
        # ==================================================================
        with (
            tc.tile_pool(name="crf", bufs=1) as crfp,
            tc.tile_pool(name="crf2", bufs=2) as crf2p,
            tc.tile_pool(name="ps_c", bufs=2, space="PSUM") as psC,
            tc.tile_pool(name="ps_c2", bufs=2, space="PSUM") as psC2,
        ):
            emis_p = statep.tile([PBJ, S], FP)
            expE = statep.tile([T, NTOK], FP)
            score_hist = statep.tile([PBJ, S], FP)
            ohtags = statep.tile([T, NTOK], FP)
            tags_sb = crfp.tile([1, NTOK], FP)
            nc.sync.dma_start(out=tags_sb[:], in_=io["tags_f"][:, :])
            for q in range(NTOK // 512):
                pb = psC.tile([128, 512], FP, tag="big", name="tpb")
                nc.tensor.matmul(
                    out=pb[:T, :],
                    lhsT=ones_row[:, :T],
                    rhs=tags_sb[:, q * 512 : (q + 1) * 512],
                    start=True,
                    stop=True,
                )
                nc.vector.tensor_tensor(
                    out=ohtags[:, q * 512 : (q + 1) * 512],
                    in0=pb[:T, :],
                    in1=iotaf[:T, :].to_broadcast([T, 512]),
                    op=OP.is_equal,
                )

            # emis regroup to [(j,b), s] via DRAM bounce (per-b strided writes)
            emv = emisT[:].rearrange("j (s b) -> j s b", b=BL)
            for b in range(BL):
                nc.sync.dma_start(
                    out=io["emis_d"][:, b, :], in_=emv[:, :, b]
                )
            nc.sync.dma_start(
                out=emis_p[:],
                in_=io["emis_d"][:, :, :].rearrange("j b s -> (j b) s"),
            )
            nc.scalar.activation(out=expE[:], in_=emisT[:], func=AF.Exp)

            P_sb = crfp.tile([PBJ, T], FP)
            nc.sync.dma_start(out=P_sb[:], in_=io["P_const"][:, :])
            Bsel_sb = crfp.tile([PBJ, PBJ], FP)
            nc.sync.dma_start(out=Bsel_sb[:], in_=io["Bsel"][:, :])
            transT_rep_sb = crfp.tile([PBJ, T], FP)
            nc.sync.dma_start(out=transT_rep_sb[:], in_=io["transT_rep"][:, :])
            expT_sb = crfp.tile([T, T], FP)
            nc.sync.dma_start(out=expT_sb[:], in_=io["expTrans"][:, :])
            expEnd_sb = crfp.tile([T, 1], FP)
            nc.sync.dma_start(out=expEnd_sb[:], in_=io["expEnd"][:, :])
            endT_sb = crfp.tile([PBJ, 1], FP)
            nc.sync.dma_start(out=endT_sb[:], in_=io["endT_rep"][:, :])
            argC_sb = crfp.tile([PBJ, T], FP)
            nc.sync.dma_start(out=argC_sb[:], in_=io["argC"][:, :])
            transT_sb = crfp.tile([T, T], FP)
            nc.sync.dma_start(out=transT_sb[:], in_=io["transT"][:, :])
            start_sb = crfp.tile([T, 1], FP)
            nc.sync.dma_start(out=start_sb[:], in_=io["start_col"][:, :])
            end_sb = crfp.tile([T, 1], FP)
            nc.sync.dma_start(out=end_sb[:], in_=io["end_col"][:, :])
            ones20 = crfp.tile([T, 1], FP)
            nc.vector.memset(ones20[:], 1.0)

            # ---- viterbi forward (scores only) ----
            startT_sb = crfp.tile([PBJ, 1], FP)
            nc.sync.dma_start(out=startT_sb[:], in_=io["startT_rep"][:, :])
            nc.vector.tensor_tensor(
                out=score_hist[:, 0:1], in0=emis_p[:, 0:1], in1=startT_sb[:],
                op=OP.add,
            )
            for u in range(1, SLS):
                sd = crf2p.tile([PBJ, T], FP, tag="sd")
                nc.vector.tensor_scalar_mul(
                    out=sd[:], in0=P_sb[:], scalar1=score_hist[:, u - 1 : u]
                )
                bc_ps = psC.tile([PBJ, T], FP, tag="bc")
                nc.tensor.matmul(
                    out=bc_ps[:], lhsT=Bsel_sb[:], rhs=sd[:], start=True,
                    stop=True,
                )
                nxt = crf2p.tile([PBJ, T], FP, tag="nxt")
                nc.vector.tensor_tensor(
                    out=nxt[:], in0=bc_ps[:], in1=transT_rep_sb[:], op=OP.add
                )
                mred = crf2p.tile([PBJ, 1], FP, tag="mred")
                nc.vector.tensor_reduce(
                    out=mred[:], in_=nxt[:], axis=AX.X, op=OP.max
                )
                nc.vector.tensor_tensor(
                    out=score_hist[:, u : u + 1], in0=mred[:],
                    in1=emis_p[:, u : u + 1], op=OP.add,
                )
            # final score+end column for host argmax
            nc.vector.tensor_tensor(
                out=score_hist[:, SLS - 1 : SLS],
                in0=score_hist[:, SLS - 1 : SLS],
                in1=endT_sb[:],
                op=OP.add,
            )
            # careful: viterbi hist recompute below uses score_hist columns
            # 0..SLS-2 only, which exclude the end-added final column.
            if debug:
                nc.sync.dma_start(out=io["dbg_score"][:, :SLS], in_=score_hist[:, :SLS])

            # ---- batched hist recompute ----
            NT1 = SLS - 1
            sdall = crfp.tile([PBJ, NT1, T], FP)
            nc.vector.tensor_tensor(
                out=sdall[:],
                in0=P_sb[:].unsqueeze(1).to_broadcast([PBJ, NT1, T]),
                in1=score_hist[:, 0:NT1].unsqueeze(2).to_broadcast([PBJ, NT1, T]),
                op=OP.mult,
            )
            nxtall = crfp.tile([PBJ, NT1, T], FP)
            TCH = 25
            for q in range((NT1 + TCH - 1) // TCH):
                t0 = q * TCH
                tn = min(TCH, NT1 - t0)
                bp = psC.tile([PBJ, 500], FP, tag="big")
                nc.tensor.matmul(
                    out=bp[:, : tn * T],
                    lhsT=Bsel_sb[:],
                    rhs=sdall[:, t0 : t0 + tn, :],
                    start=True,
                    stop=True,
                )
                nc.vector.tensor_tensor(
                    out=nxtall[:, t0 : t0 + tn, :],
                    in0=bp[:, : tn * T].rearrange("p (a b) -> p a b", b=T),
                    in1=transT_rep_sb[:].unsqueeze(1).to_broadcast([PBJ, tn, T]),
                    op=OP.add,
                )
            mall = crfp.tile([PBJ, NT1], FP)
            nc.vector.tensor_reduce(
                out=mall[:], in_=nxtall[:], axis=AX.X, op=OP.max
            )
            # eq -> (argC - BIG*eq) -> min = first argmax
            nc.vector.tensor_tensor(
                out=nxtall[:], in0=nxtall[:],
                in1=mall[:].unsqueeze(2).to_broadcast([PBJ, NT1, T]),
                op=OP.is_equal,
            )
            nc.vector.tensor_scalar_mul(
                out=nxtall[:], in0=nxtall[:], scalar1=BIG
            )
            nc.vector.tensor_tensor(
                out=nxtall[:],
                in0=argC_sb[:].unsqueeze(1).to_broadcast([PBJ, NT1, T]),
                in1=nxtall[:], op=OP.subtract,
            )
            hist_sb = crfp.tile([PBJ, NT1], FP)
            nc.vector.tensor_reduce(
                out=hist_sb[:], in_=nxtall[:], axis=AX.X, op=OP.min
            )
            nc.sync.dma_start(out=io["histv"][:, 0:NT1], in_=hist_sb[:])
            nc.sync.dma_start(
                out=io["histv"][:, S - 1 : S], in_=score_hist[:, SLS - 1 : SLS]
            )

            # ---- logsumexp forward in exp domain ----
            NBANK = (SLS - 2) // RENORM_EVERY + 2
            bank = crfp.tile([1, NBANK, BL], FP)
            nc.vector.memset(bank[:], 1.0)
            p_cur = crfp.tile([T, BL], FP, tag="pcur")
            nc.scalar.activation(
                out=p_cur[:], in_=emisT[:, 0:BL], func=AF.Exp, bias=start_sb[:]
            )
            nbank = 0
            for u in range(1, SLS):
                q_ps = psC2.tile([T, BL], FP, tag="qrb")
                nc.tensor.matmul(
                    out=q_ps[:], lhsT=expT_sb[:], rhs=p_cur[:], start=True,
                    stop=True,
                )
                nc.vector.tensor_tensor(
                    out=p_cur[:], in0=q_ps[:],
                    in1=expE[:, u * BL : (u + 1) * BL], op=OP.mult,
                )
                if u % RENORM_EVERY == 0:
                    s_ps = psC2.tile([1, BL * 3], FP, tag="s", name="sps")[:, :BL]
                    nc.tensor.matmul(
                        out=s_ps[:], lhsT=ones20[:], rhs=p_cur[:], start=True,
                        stop=True,
                    )
                    nc.vector.tensor_copy(
                        out=bank[:, nbank, :], in_=s_ps[:]
                    )
                    rc = crf2p.tile([1, BL], FP, tag="rc")
                    nc.vector.reciprocal(out=rc[:], in_=s_ps[:])
                    rb_ps = psC2.tile([T, BL], FP, tag="qrb")
                    nc.tensor.matmul(
                        out=rb_ps[:], lhsT=ones_row[:, :T],
                        rhs=rc[:], start=True, stop=True,
                    )
                    nc.vector.tensor_tensor(
                        out=p_cur[:], in0=p_cur[:], in1=rb_ps[:], op=OP.mult
                    )
                    nbank += 1
            # end transitions
            nc.vector.tensor_tensor(
                out=p_cur[:], in0=p_cur[:],
                in1=expEnd_sb[:].to_broadcast([T, BL]), op=OP.mult,
            )
            s_ps = psC2.tile([1, BL * 3], FP, tag="s", name="sps")[:, :BL]
            nc.tensor.matmul(
                out=s_ps[:], lhsT=ones20[:], rhs=p_cur[:], start=True, stop=True
            )
            nc.vector.tensor_copy(out=bank[:, nbank, :], in_=s_ps[:])
            nbank += 1
            logb = crfp.tile([1, NBANK, BL], FP)
            nc.scalar.activation(out=logb[:], in_=bank[:], func=AF.Ln)
            den = crfp.tile([1, BL], FP)
            nc.vector.tensor_reduce(
                out=den[:],
                in_=logb[:].rearrange("a n b -> a b n"),
                axis=AX.X,
                op=OP.add,
            )
            if debug:
                nc.sync.dma_start(out=io["dbg_den"][:, :], in_=den[:])
                nc.sync.dma_start(out=io["dbg_bank"][:, :, :], in_=bank[:])

            # ---- numerator ----
            # trans term: sum_t trans[tag_t, tag_{t+1}]
            npairs = (SLS - 1) * BL
            tsum = crfp.tile([T, SLS - 1, BL], FP)
            for q in range((npairs + 499) // 500):
                ps = psC.tile([PBJ, 500], FP, tag="big", name="ntr")[:T, :]
                c0 = q * 500
                cn = min(500, npairs - c0)
                nc.tensor.matmul(
                    out=ps[:, :cn], lhsT=transT_sb[:],
                    rhs=ohtags[:, BL + c0 : BL + c0 + cn], start=True, stop=True,
                )
                nc.vector.tensor_tensor(
                    out=tsum[:].rearrange("p a b -> p (a b)")[:, c0 : c0 + cn],
                    in0=ps[:, :cn],
                    in1=ohtags[:, c0 : c0 + cn],
                    op=OP.mult,
                )
            # emission term: sum_t e_t[b, tag_t]
            esum = crfp.tile([T, SLS, BL], FP)
            nc.vector.tensor_tensor(
                out=esum[:].rearrange("p a b -> p (a b)"),
                in0=emisT[:, : SLS * BL],
                in1=ohtags[:, : SLS * BL],
                op=OP.mult,
            )
            # start/end terms
            se = crfp.tile([T, 2, BL], FP)
            nc.vector.tensor_tensor(
                out=se[:, 0, :], in0=ohtags[:, 0:BL],
                in1=start_sb[:].to_broadcast([T, BL]), op=OP.mult,
            )
            nc.vector.tensor_tensor(
                out=se[:, 1, :], in0=ohtags[:, (SLS - 1) * BL : SLS * BL],
                in1=end_sb[:].to_broadcast([T, BL]), op=OP.mult,
            )
            # reduce over time (strided views -> [T, BL]), then over tags via
            # accumulating ones-matmuls into one psum
            tred = crfp.tile([T, 3, BL], FP)
            nc.vector.tensor_reduce(
                out=tred[:, 0, :],
                in_=tsum[:].rearrange("p a b -> p b a"),
                axis=AX.X, op=OP.add,
            )
            nc.vector.tensor_reduce(
                out=tred[:, 1, :],
                in_=esum[:].rearrange("p a b -> p b a"),
                axis=AX.X, op=OP.add,
            )
            nc.vector.tensor_reduce(
                out=tred[:, 2, :],
                in_=se[:].rearrange("p a b -> p b a"),
                axis=AX.X, op=OP.add,
            )
            num_ps = psC2.tile([1, BL * 3], FP, tag="s", name="nps")[:, : 3 * BL]
            nc.tensor.matmul(
                out=num_ps[:],
                lhsT=ones20[:],
                rhs=tred[:].rearrange("p a b -> p (a b)"),
                start=True,
                stop=True,
            )
            numt = crfp.tile([1, 3, BL], FP)
            nc.vector.tensor_copy(
                out=numt[:].rearrange("p a b -> p (a b)"), in_=num_ps[:]
            )
            num = crfp.tile([1, BL], FP)
            nc.vector.tensor_reduce(
                out=num[:], in_=numt[:].rearrange("p a b -> p b a"),
                axis=AX.X, op=OP.add,
            )
            if debug:
                nc.sync.dma_start(out=io["dbg_num"][:, :], in_=num[:])
            l4 = crfp.tile([1, BL], FP)
            nc.vector.tensor_tensor(
                out=l4[:], in0=den[:], in1=num[:], op=OP.subtract
            )
            lsum = crfp.tile([1, 1], FP)
            nc.vector.tensor_reduce(out=lsum[:], in_=l4[:], axis=AX.X, op=OP.add)
            nc.sync.dma_start(out=io["loss_part"][:, :], in_=lsum[:])


# ---------------------------------------------------------------------------
# host wrapper
# ---------------------------------------------------------------------------

_CACHE = {}


def _get_nc(debug=False):
    key = bool(debug)
    if key not in _CACHE:
        _CACHE[key] = build_program(debug)
    return _CACHE[key]


def run_device(inputs, debug=False, trace=False):
    nc = _get_nc(debug)
    shared = prep_shared(inputs)
    in_maps = []
    for c in range(NC):
        m = dict(shared)
        m.update(prep_core(inputs, c))
        in_maps.append(m)
    res = run_bass_kernel_spmd(
        nc, in_maps, core_ids=list(range(NC)), trace=trace
    )
    return res


def assemble_outputs(res):
    decode = np.zeros((B, S), np.int32)
    attnw = np.zeros((B, S, S), np.float32)
    loss = np.float32(0.0)
    for c in range(NC):
        r = res.results[c]
        b0 = c * BL
        aw = r["attnw"]                        # [BL, kp, kh, q]
        attnw[b0 : b0 + BL] = np.ascontiguousarray(
            aw.transpose(0, 3, 2, 1).reshape(BL, S, S)
        )
        loss += r["loss_part"][0, 0]
        hv = r["histv"].reshape(T, BL, S)      # [(j,b), s]
        fin = hv[:, :, S - 1]                  # [T, BL] final score + end
        hist = np.rint(hv[:, :, : S - 1]).astype(np.int32)  # [T, BL, S-1]
        last = np.argmax(fin, axis=0)          # [BL]
        dec = np.zeros((BL, S), np.int32)
        dec[:, S - 1] = last
        cur = last
        for t in range(S - 2, -1, -1):
            cur = hist[cur, np.arange(BL), t]
            dec[:, t] = cur
        decode[b0 : b0 + BL] = dec
    return decode, loss, attnw


def kernel(**inputs):
    res = run_device(inputs, debug=False, trace=False)
    return assemble_outputs(res)


# revision 17
# speedup vs baseline: 1.0001x; 1.0001x over previous
"""Trainium2 Bass kernel for nn_BiLSTM_18184891531327.

word-emb + grouped char-CNN -> 2-layer BiLSTM -> MHA self-attention -> CRF
(viterbi decode + NLL loss + head-averaged attention weights).

Sharding: 8 identical SPMD programs, each core owns 4 of the 32 batch
elements and computes both LSTM directions locally (the backward chain simply
indexes time steps in descending order inside the fully-unrolled loop).

Numerics tricks:
- sigmoid(x) = (tanh(x/2)+1)/2: i/f/o rows of every W/b are pre-scaled by 0.5
  on the host so one tanh activation covers all 4 gates; gate order is
  permuted to [i,f,o,g].
- h is stored as 2*h; every weight consuming h is pre-scaled by 0.5.
- CRF forward (logsumexp) runs in the exp domain as a [20,20] matmul chain
  with periodic renormalization; logs of the banked normalizers are summed at
  the end.
- Viterbi history (argmaxes) is recomputed in one batched pass from the
  stored score history; backtracking runs on the host (pointer-chasing over
  8k ints).
"""

import numpy as np

import concourse.bass as bass
import concourse.mybir as mybir
import concourse.tile as tile
from concourse.vector_clock import ScopedClock
from concourse.bass_utils import run_bass_kernel_spmd
from concourse.masks import make_identity

FP = mybir.dt.float32
AX = mybir.AxisListType
OP = mybir.AluOpType
AF = mybir.ActivationFunctionType

V, EW, EC, NF, KW = 50001, 300, 32, 4, 3
FOUT = EC * NF          # 128
H = 256
T = 20
S, B, W = 256, 32, 16
NH, E2 = 8, 512
DH = E2 // NH           # 64
NC = 8
BL = B // NC            # 4
NTOK = S * BL           # 1024, token t = s*BL + b
NW = NTOK * W
EWP = 320               # padded word-emb row
G4 = 4 * H              # 1024
NBLK = G4 // 128        # 8
PBJ = T * BL            # 80 partitions (j-major: p = j*BL + b)
BIG = 1.0e6
RENORM_EVERY = 6

# ---------------------------------------------------------------------------
# tile drain patch: walrus here accepts only 1 sync-wait per CTRL inst.
# ---------------------------------------------------------------------------


_WSPLIT_N = [0]


def _split_multiwaits(nc):
    """walrus here allows a single sync-wait per instruction: move extra
    waits onto same-engine NoOps inserted right before the instruction."""
    for fn in nc.m.functions:
        for bb in fn.blocks:
            insts = list(bb.instructions)
            if not any(
                i.sync_info is not None
                and i.sync_info.on_wait is not None
                and len(i.sync_info.on_wait) > 1
                for i in insts
            ):
                continue
            out = []
            for inst in insts:
                si = inst.sync_info
                if (
                    si is not None
                    and si.on_wait is not None
                    and len(si.on_wait) > 1
                ):
                    waits = list(si.on_wait)
                    si.on_wait = waits[-1:]
                    for w in waits[:-1]:
                        _WSPLIT_N[0] += 1
                        nop = mybir.InstNoOp(
                            name="WSPLIT-%d" % _WSPLIT_N[0], ins=[], outs=[]
                        )
                        nop.engine = inst.engine
                        nop.sync_info = mybir.SyncInfo(
                            on_wait=[w], on_update=[]
                        )
                        nc.register_instruction(nop, overwrite=True)
                        out.append(nop)
                out.append(inst)
            bb.instructions = out


def _patched_drain_and_barrier(self, tick_clock, wait_clock):
    drain_inst = self.nc.sync.drain()
    wait_clock.add_sem_waits(
        drain_inst.ins, ScopedClock({None: tick_clock.global_clock})
    )
    si = drain_inst.ins.sync_info
    if si is not None and si.on_wait is not None and len(si.on_wait) > 1:
        waits = list(si.on_wait)
        si.on_wait = waits[:1]
        for w in waits[1:]:
            d2 = self.nc.sync.drain()
            si2 = d2.ins.sync_info
            if si2 is None:
                d2.ins.sync_info = mybir.SyncInfo(on_wait=[w], on_update=[])
            else:
                si2.on_wait = [w]
    self.nc.all_engine_barrier()
    popped = self.nc._tile_sem_poison_stack.pop()
    assert popped is self._sem_poison
    self.nc.clear_and_free_semaphores(list(self.sems.allocated().values()))
    self.nc.all_engine_barrier()


tile.TileContext._drain_and_barrier = _patched_drain_and_barrier


# ---------------------------------------------------------------------------
# host-side prep
# ---------------------------------------------------------------------------


def _gate_perm_scale(a):
    """torch gate rows [i,f,g,o] -> [i,f,o,g]; i/f/o scaled by 0.5."""
    a = np.asarray(a, np.float32)
    i, f, g, o = a[0:H], a[H : 2 * H], a[2 * H : 3 * H], a[3 * H : 4 * H]
    return np.concatenate([0.5 * i, 0.5 * f, 0.5 * o, g], axis=0)


def _chunkT(wT, chunks):
    out = np.zeros((len(chunks), 128, wT.shape[1]), np.float32)
    for ci, (s0, n) in enumerate(chunks):
        out[ci, :n] = wT[s0 : s0 + n]
    return np.ascontiguousarray(out)


def _bias_tiles(b):
    return np.ascontiguousarray(np.asarray(b, np.float32).reshape(NBLK, 128).T)


def prep_shared(inp):
    d = {}
    we = np.asarray(inp["word_emb"], np.float32)
    wep = np.zeros((V, EWP), np.float32)
    wep[:, :EW] = we
    d["word_emb_pad"] = wep

    tab = np.asarray(inp["char_emb_tab"], np.float32)   # [128, 32]
    cw = np.asarray(inp["conv_w"], np.float32)          # [128, 1, 3]
    fw = np.zeros((KW, 128, FOUT), np.float32)
    for k in range(KW):
        fw[k] = tab[:, np.arange(FOUT) // NF] * cw[None, :, 0, k]
    d["fw_conv"] = fw
    d["conv_b"] = np.asarray(inp["conv_b"], np.float32).reshape(FOUT, 1)

    l0_chunks = [(0, 128), (128, 128), (256, 44), (300, 128)]
    l1_chunks = [(0, 128), (128, 128), (256, 128), (384, 128)]
    for layer, chunks in ((0, l0_chunks), (1, l1_chunks)):
        for dr in ("f", "b"):
            sfx = "l%d%s" % (layer, dr)
            wih = _gate_perm_scale(inp["w_ih_" + sfx])
            whh = _gate_perm_scale(inp["w_hh_" + sfx]) * 0.5
            if layer == 1:
                wih = wih * 0.5
            bb = _gate_perm_scale(
                (
                    np.asarray(inp["b_ih_" + sfx], np.float32)
                    + np.asarray(inp["b_hh_" + sfx], np.float32)
                )[:, None]
            )[:, 0]
            d["wih_" + sfx] = _chunkT(np.ascontiguousarray(wih.T), chunks)
            d["whh_" + sfx] = _chunkT(
                np.ascontiguousarray(whh.T), [(0, 128), (128, 128)]
            )
            d["bias_" + sfx] = _bias_tiles(bb)

    ipw = np.asarray(inp["in_proj_w"], np.float32)
    ipb = np.asarray(inp["in_proj_b"], np.float32)
    sc = 1.0 / np.sqrt(np.float32(DH))
    ch4 = [(0, 128), (128, 128), (256, 128), (384, 128)]
    d["wqT"] = _chunkT(np.ascontiguousarray((ipw[0:E2] * 0.5 * sc).T), ch4)
    d["wkT"] = _chunkT(np.ascontiguousarray((ipw[E2 : 2 * E2] * 0.5).T), ch4)
    d["wvT"] = _chunkT(np.ascontiguousarray((ipw[2 * E2 :] * 0.5).T), ch4)
    d["bq"] = np.ascontiguousarray((ipb[0:E2] * sc).reshape(4, 128).T)
    d["bk"] = np.ascontiguousarray(ipb[E2 : 2 * E2].reshape(4, 128).T)
    d["bv"] = np.ascontiguousarray(ipb[2 * E2 :].reshape(1, E2))
    opwT = np.ascontiguousarray(np.asarray(inp["out_proj_w"], np.float32).T)
    d["woT"] = np.ascontiguousarray(opwT.reshape(NH, DH, E2))
    d["bo"] = np.ascontiguousarray(
        np.asarray(inp["out_proj_b"], np.float32).reshape(4, 128).T
    )
    d["fcT"] = _chunkT(
        np.ascontiguousarray(np.asarray(inp["fc_w"], np.float32).T), ch4
    )
    d["bfc"] = np.asarray(inp["fc_b"], np.float32).reshape(T, 1)

    trans = np.asarray(inp["trans"], np.float32)
    start = np.asarray(inp["start_trans"], np.float32)
    end = np.asarray(inp["end_trans"], np.float32)
    p_j = np.repeat(np.arange(T), BL)              # p=(j,b): j index
    d["transT_rep"] = np.ascontiguousarray(trans.T[p_j])          # [80,20]
    d["P_const"] = (p_j[:, None] == np.arange(T)[None, :]).astype(np.float32)
    bcol = np.tile(np.arange(BL), T)
    d["Bsel"] = (bcol[:, None] == bcol[None, :]).astype(np.float32)
    d["expTrans"] = np.exp(trans).astype(np.float32)
    d["expEnd"] = np.exp(end).astype(np.float32).reshape(T, 1)
    d["endT_rep"] = np.ascontiguousarray(end[p_j].reshape(PBJ, 1))
    d["startT_rep"] = np.ascontiguousarray(start[p_j].reshape(PBJ, 1))
    d["argC"] = np.ascontiguousarray(
        np.broadcast_to(np.arange(T, dtype=np.float32)[None, :] + BIG, (PBJ, T))
    )
    d["transT"] = np.ascontiguousarray(trans.T)
    d["start_col"] = start.reshape(T, 1).astype(np.float32)
    d["end_col"] = end.reshape(T, 1).astype(np.float32)
    return d


def prep_core(inp, core):
    d = {}
    b0 = core * BL
    words = np.asarray(inp["words"]).astype(np.int64)[:, b0 : b0 + BL]
    chars = np.asarray(inp["chars"]).astype(np.int64)[b0 : b0 + BL]
    tags = np.asarray(inp["tags"]).astype(np.int64)[:, b0 : b0 + BL]
    widx = words.reshape(NTOK).astype(np.int32)
    d["widx"] = np.ascontiguousarray(widx.reshape(NTOK // 128, 128))
    d["chars_f"] = np.ascontiguousarray(
        chars.transpose(1, 0, 2).reshape(1, NW).astype(np.float32)
    )
    d["tags_f"] = np.ascontiguousarray(
        tags.reshape(1, NTOK).astype(np.float32)
    )
    return d


# ---------------------------------------------------------------------------
# device program
# ---------------------------------------------------------------------------

SLS = S  # lstm/crf step count (shrink for debugging only)


def build_program(debug=False):
    nc = bass.Bass("TRN2")

    def par(name, shape, dtype=FP):
        return nc.declare_dram_parameter(name, list(shape), dtype, isOutput=False)

    def out_par(name, shape, dtype=FP):
        return nc.declare_dram_parameter(name, list(shape), dtype, isOutput=True)

    io = {}
    io["word_emb_pad"] = par("word_emb_pad", [V, EWP])
    io["fw_conv"] = par("fw_conv", [KW, 128, FOUT])
    io["conv_b"] = par("conv_b", [FOUT, 1])
    for layer in (0, 1):
        for dr in ("f", "b"):
            sfx = "l%d%s" % (layer, dr)
            io["wih_" + sfx] = par("wih_" + sfx, [4, 128, G4])
            io["whh_" + sfx] = par("whh_" + sfx, [2, 128, G4])
            io["bias_" + sfx] = par("bias_" + sfx, [128, NBLK])
    for nm, shp in (
        ("wqT", [4, 128, E2]),
        ("wkT", [4, 128, E2]),
        ("wvT", [4, 128, E2]),
        ("bq", [128, 4]),
        ("bk", [128, 4]),
        ("bv", [1, E2]),
        ("woT", [NH, DH, E2]),
        ("bo", [128, 4]),
        ("fcT", [4, 128, T]),
        ("bfc", [T, 1]),
        ("transT_rep", [PBJ, T]),
        ("P_const", [PBJ, T]),
        ("Bsel", [PBJ, PBJ]),
        ("expTrans", [T, T]),
        ("expEnd", [T, 1]),
        ("endT_rep", [PBJ, 1]),
        ("startT_rep", [PBJ, 1]),
        ("argC", [PBJ, T]),
        ("transT", [T, T]),
        ("start_col", [T, 1]),
        ("end_col", [T, 1]),
        ("chars_f", [1, NW]),
        ("tags_f", [1, NTOK]),
    ):
        io[nm] = par(nm, shp)
    io["widx"] = par("widx", [NTOK // 128, 128], mybir.dt.int32)

    io["attnw"] = out_par("attnw", [BL, 128, 2, S])  # (b, kpart, khalf, q)
    io["histv"] = out_par("histv", [PBJ, S])   # cols 0..S-2 hist, col S-1 final score+end
    io["loss_part"] = out_par("loss_part", [1, 1])
    if debug:
        io["dbg_xT"] = out_par("dbg_xT", [4, 128, NTOK])
        io["dbg_gx"] = out_par("dbg_gx", [128, NBLK, NTOK])
        io["dbg_h01"] = out_par("dbg_h01", [2, 128, 4, NTOK])
        io["dbg_qkv"] = out_par("dbg_qkv", [3, 128, 4096])
        io["dbg_emis"] = out_par("dbg_emis", [T, NTOK])
        io["dbg_score"] = out_par("dbg_score", [PBJ, S])
        io["dbg_den"] = out_par("dbg_den", [1, BL])
        io["dbg_bank"] = out_par("dbg_bank", [1, (S - 2) // RENORM_EVERY + 2, BL])
        io["dbg_num"] = out_par("dbg_num", [1, BL])

    io["emis_d"] = nc.dram_tensor("emis_d", [T, BL, S], FP)

    with tile.TileContext(nc) as tc:
        _body(nc, tc, io, debug)
    _split_multiwaits(nc)
    return nc


def _body(nc, tc, io, debug):
    MI32 = mybir.dt.int32

    with (
        tc.tile_pool(name="const", bufs=1) as constp,
        tc.tile_pool(name="state", bufs=1) as statep,
    ):
        ident = constp.tile([128, 128], FP)
        make_identity(nc, ident[:])
        iota_i = constp.tile([128, 1], MI32)
        nc.gpsimd.iota(iota_i[:], pattern=[[0, 1]], base=0, channel_multiplier=1)
        iotaf = constp.tile([128, 1], FP)
        nc.vector.tensor_copy(out=iotaf[:], in_=iota_i[:])
        ones_row = constp.tile([1, 128], FP)
        nc.vector.memset(ones_row[:], 1.0)
        ones_col128 = constp.tile([128, 1], FP)
        nc.vector.memset(ones_col128[:], 1.0)


        # ==================================================================
        # Phase A: embeddings + char conv -> xT [128, 4, NTOK]
        # (chunks: wd 0:128, wd 128:256, wd 256:300 (44 rows), char 128)
        # ==================================================================
        with (
            tc.tile_pool(name="xT", bufs=1) as xTp,
            tc.tile_pool(name="wtok", bufs=3) as wtokp,
            tc.tile_pool(name="oh", bufs=1) as ohp,
            tc.tile_pool(name="wih", bufs=1) as wihp,
            tc.tile_pool(name="gx", bufs=1) as gxp,
            tc.tile_pool(name="misc_a", bufs=2) as miscp,
            tc.tile_pool(name="ps_a", bufs=2, space="PSUM") as psA,
            tc.tile_pool(name="ps_g", bufs=4, space="PSUM") as psG,
        ):
            xT = xTp.tile([128, 4, NTOK], FP)
            nc.vector.memset(xT[:, 2, :], 0.0)

            widx_sb = constp.tile([128, NTOK // 128], MI32)
            nc.sync.dma_start(
                out=widx_sb[:], in_=io["widx"][:, :].rearrange("j p -> p j")
            )
            for j in range(NTOK // 128):
                wtok = wtokp.tile([128, EWP], FP, tag="wtok")
                nc.gpsimd.indirect_dma_start(
                    out=wtok[:],
                    out_offset=None,
                    in_=io["word_emb_pad"][:, :],
                    in_offset=bass.IndirectOffsetOnAxis(
                        ap=widx_sb[:, j : j + 1], axis=0
                    ),
                )
                for ci, (c0, n) in enumerate([(0, 128), (128, 128), (256, 44)]):
                    pt = psA.tile([128, 512], FP, tag="bc", name="tp")[:, :128]
                    nc.tensor.transpose(
                        out=pt[:n, :], in_=wtok[:, c0 : c0 + n], identity=ident[:]
                    )
                    nc.vector.tensor_copy(
                        out=xT[:n, ci, j * 128 : (j + 1) * 128], in_=pt[:n, :]
                    )

            # char one-hot + fused conv + maxpool -> xT chunk 3
            fw_sb = constp.tile([128, KW, FOUT], FP)
            nc.sync.dma_start(
                out=fw_sb[:], in_=io["fw_conv"][:, :, :].rearrange("k p f -> p k f")
            )
            convb_sb = constp.tile([128, 1], FP)
            nc.sync.dma_start(out=convb_sb[:], in_=io["conv_b"][:, :])

            SEG = 128   # tokens per one-hot segment
            CH = 32     # tokens per conv chunk
            PW = W - KW + 1  # 14
            for seg in range(NTOK // SEG):
                chseg = miscp.tile([1, SEG * W], FP, tag="chseg", bufs=1)
                nc.sync.dma_start(
                    out=chseg[:],
                    in_=io["chars_f"][:, seg * SEG * W : (seg + 1) * SEG * W],
                )
                oh = ohp.tile([128, SEG, W], FP, tag="oh")
                for q in range(SEG * W // 512):
                    pb = psA.tile([128, 512], FP, tag="bc")
                    nc.tensor.matmul(
                        out=pb[:],
                        lhsT=ones_row[:],
                        rhs=chseg[:, q * 512 : (q + 1) * 512],
                        start=True,
                        stop=True,
                    )
                    nc.vector.tensor_tensor(
                        out=oh[:].rearrange("p a b -> p (a b)")[
                            :, q * 512 : (q + 1) * 512
                        ],
                        in0=pb[:],
                        in1=iotaf[:].to_broadcast([128, 512]),
                        op=OP.is_equal,
                    )
                for c in range(SEG // CH):
                    pc = psA.tile([128, CH, PW], FP, tag="conv")
                    for k in range(KW):
                        nc.tensor.matmul(
                            out=pc[:],
                            lhsT=fw_sb[:, k, :],
                            rhs=oh[:, c * CH : (c + 1) * CH, k : k + PW],
                            start=(k == 0),
                            stop=(k == KW - 1),
                        )
                    mx = miscp.tile([128, CH], FP, tag="mx")
                    nc.vector.tensor_reduce(
                        out=mx[:], in_=pc[:], axis=AX.X, op=OP.max
                    )
                    nc.vector.tensor_scalar_add(
                        out=xT[:, 3, seg * SEG + c * CH :][:, :CH],
                        in0=mx[:],
                        scalar1=convb_sb[:],
                    )
            if debug:
                nc.sync.dma_start(
                    out=io["dbg_xT"][:, :, :],
                    in_=xT[:].rearrange("p c n -> c p n"),
                )

            # ==============================================================
            # gates_x + LSTM, both layers, both directions
            # ==============================================================
            h0_all = statep.tile([128, 4, NTOK], FP)  # chunks [f0,f1,b0,b1]
            whh_sb = {}
            bias_sb = {}
            for layer in (0, 1):
                for dr in ("f", "b"):
                    sfx = "l%d%s" % (layer, dr)
                    whh_sb[sfx] = wihp.tile(
                        [128, 2, G4], FP, tag="whh_" + sfx, name="whh_" + sfx
                    )
                    nc.sync.dma_start(
                        out=whh_sb[sfx][:],
                        in_=io["whh_" + sfx][:, :, :].rearrange("k p g -> p k g"),
                    )
                    bias_sb[sfx] = constp.tile(
                        [128, NBLK], FP, tag="bias_" + sfx, name="bias_" + sfx
                    )
                    nc.sync.dma_start(
                        out=bias_sb[sfx][:], in_=io["bias_" + sfx][:, :]
                    )

            gx = {
                "f": gxp.tile([128, NBLK, NTOK], FP, tag="gxf", name="gxf"),
                "b": gxp.tile([128, NBLK, NTOK], FP, tag="gxb", name="gxb"),
            }

            def gates_x(layer, dr, rhs_chunks):
                sfx = "l%d%s" % (layer, dr)
                wt = wihp.tile([128, 4, G4], FP, tag="wih")
                nc.sync.dma_start(
                    out=wt[:],
                    in_=io["wih_" + sfx][:, :, :].rearrange("k p g -> p k g"),
                )
                for m in range(NBLK):
                    for nch in range(NTOK // 512):
                        ps = psG.tile([128, 512], FP, tag="g")
                        for kc in range(4):
                            nc.tensor.matmul(
                                out=ps[:],
                                lhsT=wt[:, kc, m * 128 : (m + 1) * 128],
                                rhs=rhs_chunks(kc, nch),
                                start=(kc == 0),
                                stop=(kc == 3),
                            )
                        nc.vector.tensor_scalar_add(
                            out=gx[dr][:, m, nch * 512 :][:, :512],
                            in0=ps[:],
                            scalar1=bias_sb[sfx][:, m : m + 1],
                        )

            def lstm_chain(layer, dr, h_dst):
                """h_dst: [128, 2, NTOK] view of h{layer}_all chunks."""
                sfx = "l%d%s" % (layer, dr)
                c_t = statep.tile([128, 2, BL], FP, tag="c_" + sfx)
                nc.vector.memset(c_t[:], 0.0)
                for u in range(SLS):
                    s = u if dr == "f" else SLS - 1 - u
                    gxs = gx[dr][:, :, s * BL : (s + 1) * BL]   # [128,8,BL]
                    if u == 0:
                        gsrc = gxs
                    else:
                        sp = s + 1 if dr == "b" else s - 1
                        hprev = h_dst[:, :, sp * BL : (sp + 1) * BL]
                        pse = psG.tile([128, 512], FP, tag="g", name="lpsE")[
                            :, : NBLK * BL // 2
                        ]
                        pso = psG.tile([128, 512], FP, tag="g", name="lpsO")[
                            :, : NBLK * BL // 2
                        ]
                        for m in range(NBLK):
                            tgt = pse if m < 4 else pso
                            col = (m % 4) * BL
                            for kc in range(2):
                                nc.tensor.matmul(
                                    out=tgt[:, col : col + BL],
                                    lhsT=whh_sb[sfx][:, kc, m * 128 :][:, :128],
                                    rhs=hprev[:, kc, :],
                                    start=(kc == 0),
                                    stop=(kc == 1),
                                )
                        gfull = miscp.tile([128, NBLK, BL], FP, tag="gf" + dr)
                        nc.vector.tensor_tensor(
                            out=gfull[:, 0:4, :],
                            in0=pse[:].rearrange("p (m b) -> p m b", m=4),
                            in1=gxs[:, 0:4, :], op=OP.add,
                        )
                        nc.vector.tensor_tensor(
                            out=gfull[:, 4:8, :],
                            in0=pso[:].rearrange("p (m b) -> p m b", m=4),
                            in1=gxs[:, 4:8, :], op=OP.add,
                        )
                        gsrc = gfull[:]
                    tall = miscp.tile([128, NBLK, BL], FP, tag="ta" + dr)
                    nc.scalar.activation(out=tall[:], in_=gsrc, func=AF.Tanh)
                    ti = tall[:, 0:2, :]
                    tf = tall[:, 2:4, :]
                    to = tall[:, 4:6, :]
                    tg = tall[:, 6:8, :]
                    u1 = miscp.tile([128, 2, BL], FP, tag="u1" + dr)
                    nc.vector.scalar_tensor_tensor(
                        out=u1[:], in0=tf, scalar=1.0, in1=c_t[:],
                        op0=OP.add, op1=OP.mult,
                    )
                    v1 = miscp.tile([128, 2, BL], FP, tag="v1" + dr)
                    nc.vector.scalar_tensor_tensor(
                        out=v1[:], in0=ti, scalar=1.0, in1=tg,
                        op0=OP.add, op1=OP.mult,
                    )
                    x2 = miscp.tile([128, 2, BL], FP, tag="x2" + dr)
                    nc.vector.tensor_tensor(
                        out=x2[:], in0=u1[:], in1=v1[:], op=OP.add
                    )
                    tc2 = miscp.tile([128, 2, BL], FP, tag="tc" + dr)
                    nc.scalar.activation(
                        out=tc2[:], in_=x2[:], func=AF.Tanh, scale=0.5
                    )
                    nc.vector.tensor_scalar_mul(
                        out=c_t[:], in0=x2[:], scalar1=0.5
                    )
                    nc.vector.scalar_tensor_tensor(
                        out=h_dst[:, :, s * BL : (s + 1) * BL],
                        in0=to, scalar=1.0, in1=tc2[:],
                        op0=OP.add, op1=OP.mult,
                    )

            # layer 0
            gates_x(0, "f", lambda kc, nch: xT[:, kc, nch * 512 :][:, :512])
            gates_x(0, "b", lambda kc, nch: xT[:, kc, nch * 512 :][:, :512])
            if debug:
                nc.sync.dma_start(out=io["dbg_gx"][:, :, :], in_=gx["f"][:])
            lstm_chain(0, "f", h0_all[:, 0:2, :])
            lstm_chain(0, "b", h0_all[:, 2:4, :])
            # layer 1 (input = h0_all, stored doubled; wih_l1 pre-halved)
            h1_all = statep.tile([128, 4, NTOK], FP)
            gates_x(1, "f", lambda kc, nch: h0_all[:, kc, nch * 512 :][:, :512])
            gates_x(1, "b", lambda kc, nch: h0_all[:, kc, nch * 512 :][:, :512])
            lstm_chain(1, "f", h1_all[:, 0:2, :])
            lstm_chain(1, "b", h1_all[:, 2:4, :])
            if debug:
                nc.sync.dma_start(
                    out=io["dbg_h01"][0, :, :, :], in_=h0_all[:]
                )
                nc.sync.dma_start(
                    out=io["dbg_h01"][1, :, :, :], in_=h1_all[:]
                )

        # ==================================================================
        # Phase F: attention + fc -> emisT [20, NTOK]
        # ==================================================================
        with (
            tc.tile_pool(name="att", bufs=1) as attp,
            tc.tile_pool(name="attw", bufs=2) as attwp,
            tc.tile_pool(name="et", bufs=3) as etp,
            tc.tile_pool(name="ps_f", bufs=3, space="PSUM") as psF,
            tc.tile_pool(name="ps_f2", bufs=1, space="PSUM") as psF2,
        ):
            emisT = statep.tile([T, NTOK], FP)
            qT = attp.tile([128, 4, NTOK], FP)
            kT = attp.tile([128, 4, NTOK], FP)
            vv = attp.tile([128, NBLK, E2], FP)     # [s-half|b] token tiles
            av_all = attp.tile([DH, NH, S, BL], FP)
            attnT = attp.tile([128, 4, NTOK], FP)

            wq_sb = attwp.tile([128, 4, E2], FP, tag="wstream")
            nc.sync.dma_start(
                out=wq_sb[:], in_=io["wqT"][:, :, :].rearrange("k p g -> p k g")
            )
            wk_sb = attwp.tile([128, 4, E2], FP, tag="wstream")
            nc.sync.dma_start(
                out=wk_sb[:], in_=io["wkT"][:, :, :].rearrange("k p g -> p k g")
            )
            wv_sb = attwp.tile([128, 4, E2], FP, tag="wstream")
            nc.sync.dma_start(
                out=wv_sb[:], in_=io["wvT"][:, :, :].rearrange("k p g -> p k g")
            )
            bq_sb = constp.tile([128, 4], FP)
            nc.sync.dma_start(out=bq_sb[:], in_=io["bq"][:, :])
            bk_sb = constp.tile([128, 4], FP)
            nc.sync.dma_start(out=bk_sb[:], in_=io["bk"][:, :])
            bv_sb = constp.tile([128, E2], FP)
            nc.sync.dma_start(
                out=bv_sb[:], in_=io["bv"][:, :].to_broadcast([128, E2])
            )

            for dst, wsb, bsb in ((qT, wq_sb, bq_sb), (kT, wk_sb, bk_sb)):
                for m in range(4):
                    for nch in range(NTOK // 512):
                        ps = psF.tile([128, 512], FP, tag="f")
                        for kc in range(4):
                            nc.tensor.matmul(
                                out=ps[:],
                                lhsT=wsb[:, kc, m * 128 : (m + 1) * 128],
                                rhs=h1_all[:, kc, nch * 512 :][:, :512],
                                start=(kc == 0),
                                stop=(kc == 3),
                            )
                        nc.vector.tensor_scalar_add(
                            out=dst[:, m, nch * 512 :][:, :512],
                            in0=ps[:],
                            scalar1=bsb[:, m : m + 1],
                        )
            # v, token-major, tiles indexed (b, s-half)
            h1v = h1_all[:].rearrange("p c (s b) -> p c s b", b=BL)
            for b in range(BL):
                for kh in range(2):
                    ps = psF.tile([128, E2], FP, tag="f")
                    for kc in range(4):
                        nc.tensor.matmul(
                            out=ps[:],
                            lhsT=h1v[:, kc, kh * 128 : (kh + 1) * 128, b],
                            rhs=wv_sb[:, kc, :],
                            start=(kc == 0),
                            stop=(kc == 3),
                        )
                    nc.vector.tensor_tensor(
                        out=vv[:, b * 2 + kh, :], in0=ps[:], in1=bv_sb[:],
                        op=OP.add,
                    )
            if debug:
                nc.sync.dma_start(
                    out=io["dbg_qkv"][0], in_=qT[:].rearrange("p c n -> p (c n)")
                )
                nc.sync.dma_start(
                    out=io["dbg_qkv"][1], in_=kT[:].rearrange("p c n -> p (c n)")
                )
                nc.sync.dma_start(
                    out=io["dbg_qkv"][2], in_=vv[:].rearrange("p t e -> p (t e)")
                )

            qTv = qT[:].rearrange("p c (s b) -> p c s b", b=BL)
            kTv = kT[:].rearrange("p c (s b) -> p c s b", b=BL)
            for b in range(BL):
                awacc = attwp.tile([128, 2, S], FP, tag="awacc")
                for h in range(NH):
                    ch, off = h // 2, (h % 2) * DH
                    q_bh = qTv[off : off + DH, ch, :, b]    # [64, 256]
                    k_bh = kTv[off : off + DH, ch, :, b]
                    et = etp.tile([128, 2, S], FP, tag="et")
                    rs_ps = psF2.tile([1, S], FP, tag="rs")
                    for kh in range(2):
                        sc_ps = psF.tile([128, 512], FP, tag="f", name="scps")[:, :S]
                        nc.tensor.matmul(
                            out=sc_ps[:],
                            lhsT=k_bh[:, kh * 128 : (kh + 1) * 128],
                            rhs=q_bh,
                            start=True,
                            stop=True,
                        )
                        nc.scalar.activation(
                            out=et[:, kh, :], in_=sc_ps[:], func=AF.Exp
                        )
                    for kh in range(2):
                        nc.tensor.matmul(
                            out=rs_ps[:],
                            lhsT=ones_col128[:],
                            rhs=et[:, kh, :],
                            start=(kh == 0),
                            stop=(kh == 1),
                        )
                    rq = etp.tile([1, S], FP, tag="rq")
                    nc.vector.reciprocal(out=rq[:], in_=rs_ps[:])
                    # av_raw^T [64, 256] accumulated over k halves
                    av_ps = psF2.tile([DH, S], FP, tag="av")
                    for kh in range(2):
                        nc.tensor.matmul(
                            out=av_ps[:],
                            lhsT=vv[:, b * 2 + kh, h * DH : (h + 1) * DH],
                            rhs=et[:, kh, :],
                            start=(kh == 0),
                            stop=(kh == 1),
                        )
                    rqb_ps = psF2.tile([DH, S], FP, tag="rqb")
                    nc.tensor.matmul(
                        out=rqb_ps[:],
                        lhsT=ones_row[:, :DH],
                        rhs=rq[:],
                        start=True,
                        stop=True,
                    )
                    rqb = etp.tile([DH, S], FP, tag="rqbs")
                    nc.vector.tensor_copy(out=rqb[:], in_=rqb_ps[:])
                    nc.vector.tensor_tensor(
                        out=av_all[:, h, :, b], in0=av_ps[:], in1=rqb[:],
                        op=OP.mult,
                    )
                    # attn-weight accumulation: awacc (+)= et * rq_bc128
                    rqb128_ps = psF2.tile([128, S], FP, tag="rqb128")
                    nc.tensor.matmul(
                        out=rqb128_ps[:],
                        lhsT=ones_row[:],
                        rhs=rq[:],
                        start=True,
                        stop=True,
                    )
                    anorm = etp.tile([128, 2, S], FP, tag="anorm")
                    for kh in range(2):
                        nc.vector.tensor_tensor(
                            out=anorm[:, kh, :], in0=rqb128_ps[:],
                            in1=et[:, kh, :], op=OP.mult,
                        )
                    if h == 0:
                        nc.vector.tensor_scalar_mul(
                            out=awacc[:], in0=anorm[:], scalar1=1.0 / NH
                        )
                    else:
                        nc.vector.scalar_tensor_tensor(
                            out=awacc[:], in0=anorm[:], scalar=1.0 / NH,
                            in1=awacc[:], op0=OP.mult, op1=OP.add,
                        )
                nc.sync.dma_start(out=io["attnw"][b], in_=awacc[:])

            # out_proj + fc
            wo_sb = attwp.tile([DH, NH, E2], FP, tag="wstream")
            nc.sync.dma_start(
                out=wo_sb[:], in_=io["woT"][:, :, :].rearrange("h p g -> p h g")
            )
            bo_sb = constp.tile([128, 4], FP)
            nc.sync.dma_start(out=bo_sb[:], in_=io["bo"][:, :])
            for mo in range(4):
                for nch in range(NTOK // 512):
                    ps = psF.tile([128, 512], FP, tag="f")
                    for h in range(NH):
                        nc.tensor.matmul(
                            out=ps[:],
                            lhsT=wo_sb[:, h, mo * 128 : (mo + 1) * 128],
                            rhs=av_all[:, h, nch * 128 : (nch + 1) * 128, :],
                            start=(h == 0),
                            stop=(h == NH - 1),
                        )
                    nc.vector.tensor_scalar_add(
                        out=attnT[:, mo, nch * 512 :][:, :512],
                        in0=ps[:],
                        scalar1=bo_sb[:, mo : mo + 1],
                    )
            fc_sb = attwp.tile([128, 4, T], FP, tag="fcw")
            nc.sync.dma_start(
                out=fc_sb[:], in_=io["fcT"][:, :, :].rearrange("k p t -> p k t")
            )
            bfc_sb = constp.tile([T, 1], FP)
            nc.sync.dma_start(out=bfc_sb[:], in_=io["bfc"][:, :])
            for nch in range(NTOK // 512):
                ps = psF.tile([128, 512], FP, tag="f", name="emps")[:T, :]
                for kc in range(4):
                    nc.tensor.matmul(
                        out=ps[:],
                        lhsT=fc_sb[:, kc, :],
                        rhs=attnT[:, kc, nch * 512 :][:, :512],
                        start=(kc == 0),
                        stop=(kc == 3),
                    )
                nc.vector.tensor_scalar_add(
                    out=emisT[:, nch * 512 : (nch + 1) * 512],
                    in0=ps[:],
                    scalar1=bfc_sb[:],
                )
            if debug:
                nc.sync.dma_start(out=io["dbg_emis"][:, :], in_=emisT[:])

        # ==================================================================
        # Phase G: CRF
        # ==================================================================
        with (
            tc.tile_pool(name="crf", bufs=1) as crfp,
            tc.tile_pool(name="crf2", bufs=2) as crf2p,
            tc.tile_pool(name="ps_c", bufs=2, space="PSUM") as psC,
            tc.tile_pool(name="ps_c2", bufs=2, space="PSUM") as psC2,
        ):
            emis_p = statep.tile([PBJ, S], FP)
            expE = statep.tile([T, NTOK], FP)
            score_hist = statep.tile([PBJ, S], FP)
            ohtags = statep.tile([T, NTOK], FP)
            tags_sb = crfp.tile([1, NTOK], FP)
            nc.sync.dma_start(out=tags_sb[:], in_=io["tags_f"][:, :])
            for q in range(NTOK // 512):
                pb = psC.tile([128, 512], FP, tag="big", name="tpb")
                nc.tensor.matmul(
                    out=pb[:T, :],
                    lhsT=ones_row[:, :T],
                    rhs=tags_sb[:, q * 512 : (q + 1) * 512],
                    start=True,
                    stop=True,
                )
                nc.vector.tensor_tensor(
                    out=ohtags[:, q * 512 : (q + 1) * 512],
                    in0=pb[:T, :],
                    in1=iotaf[:T, :].to_broadcast([T, 512]),
                    op=OP.is_equal,
                )

            # emis regroup to [(j,b), s] via DRAM bounce (per-b strided writes)
            emv = emisT[:].rearrange("j (s b) -> j s b", b=BL)
            for b in range(BL):
                nc.sync.dma_start(
                    out=io["emis_d"][:, b, :], in_=emv[:, :, b]
                )
            nc.sync.dma_start(
                out=emis_p[:],
                in_=io["emis_d"][:, :, :].rearrange("j b s -> (j b) s"),
            )
            nc.scalar.activation(out=expE[:], in_=emisT[:], func=AF.Exp)

            P_sb = crfp.tile([PBJ, T], FP)
            nc.sync.dma_start(out=P_sb[:], in_=io["P_const"][:, :])
            Bsel_sb = crfp.tile([PBJ, PBJ], FP)
            nc.sync.dma_start(out=Bsel_sb[:], in_=io["Bsel"][:, :])
            transT_rep_sb = crfp.tile([PBJ, T], FP)
            nc.sync.dma_start(out=transT_rep_sb[:], in_=io["transT_rep"][:, :])
            expT_sb = crfp.tile([T, T], FP)
            nc.sync.dma_start(out=expT_sb[:], in_=io["expTrans"][:, :])
            expEnd_sb = crfp.tile([T, 1], FP)
            nc.sync.dma_start(out=expEnd_sb[:], in_=io["expEnd"][:, :])
            endT_sb = crfp.tile([PBJ, 1], FP)
            nc.sync.dma_start(out=endT_sb[:], in_=io["endT_rep"][:, :])
            argC_sb = crfp.tile([PBJ, T], FP)
            nc.sync.dma_start(out=argC_sb[:], in_=io["argC"][:, :])
            transT_sb = crfp.tile([T, T], FP)
            nc.sync.dma_start(out=transT_sb[:], in_=io["transT"][:, :])
            start_sb = crfp.tile([T, 1], FP)
            nc.sync.dma_start(out=start_sb[:], in_=io["start_col"][:, :])
            end_sb = crfp.tile([T, 1], FP)
            nc.sync.dma_start(out=end_sb[:], in_=io["end_col"][:, :])
            ones20 = crfp.tile([T, 1], FP)
            nc.vector.memset(ones20[:], 1.0)

            # ---- viterbi forward (scores only) ----
            startT_sb = crfp.tile([PBJ, 1], FP)
            nc.sync.dma_start(out=startT_sb[:], in_=io["startT_rep"][:, :])
            nc.vector.tensor_tensor(
                out=score_hist[:, 0:1], in0=emis_p[:, 0:1], in1=startT_sb[:],
                op=OP.add,
            )
            for u in range(1, SLS):
                sd = crf2p.tile([PBJ, T], FP, tag="sd")
                nc.vector.tensor_scalar_mul(
                    out=sd[:], in0=P_sb[:], scalar1=score_hist[:, u - 1 : u]
                )
                bc_ps = psC.tile([PBJ, T], FP, tag="bc")
                nc.tensor.matmul(
                    out=bc_ps[:], lhsT=Bsel_sb[:], rhs=sd[:], start=True,
                    stop=True,
                )
                nxt = crf2p.tile([PBJ, T], FP, tag="nxt")
                nc.vector.tensor_tensor(
                    out=nxt[:], in0=bc_ps[:], in1=transT_rep_sb[:], op=OP.add
                )
                mred = crf2p.tile([PBJ, 1], FP, tag="mred")
                nc.vector.tensor_reduce(
                    out=mred[:], in_=nxt[:], axis=AX.X, op=OP.max
                )
                nc.vector.tensor_tensor(
                    out=score_hist[:, u : u + 1], in0=mred[:],
                    in1=emis_p[:, u : u + 1], op=OP.add,
                )
            # final score+end column for host argmax
            nc.vector.tensor_tensor(
                out=score_hist[:, SLS - 1 : SLS],
                in0=score_hist[:, SLS - 1 : SLS],
                in1=endT_sb[:],
                op=OP.add,
            )
            # careful: viterbi hist recompute below uses score_hist columns
            # 0..SLS-2 only, which exclude the end-added final column.
            if debug:
                nc.sync.dma_start(out=io["dbg_score"][:, :SLS], in_=score_hist[:, :SLS])

            # ---- batched hist recompute ----
            NT1 = SLS - 1
            sdall = crfp.tile([PBJ, NT1, T], FP)
            nc.vector.tensor_tensor(
                out=sdall[:],
                in0=P_sb[:].unsqueeze(1).to_broadcast([PBJ, NT1, T]),
                in1=score_hist[:, 0:NT1].unsqueeze(2).to_broadcast([PBJ, NT1, T]),
                op=OP.mult,
            )
            nxtall = crfp.tile([PBJ, NT1, T], FP)
            TCH = 25
            for q in range((NT1 + TCH - 1) // TCH):
                t0 = q * TCH
                tn = min(TCH, NT1 - t0)
                bp = psC.tile([PBJ, 500], FP, tag="big")
                nc.tensor.matmul(
                    out=bp[:, : tn * T],
                    lhsT=Bsel_sb[:],
                    rhs=sdall[:, t0 : t0 + tn, :],
                    start=True,
                    stop=True,
                )
                nc.vector.tensor_tensor(
                    out=nxtall[:, t0 : t0 + tn, :],
                    in0=bp[:, : tn * T].rearrange("p (a b) -> p a b", b=T),
                    in1=transT_rep_sb[:].unsqueeze(1).to_broadcast([PBJ, tn, T]),
                    op=OP.add,
                )
            mall = crfp.tile([PBJ, NT1], FP)
            nc.vector.tensor_reduce(
                out=mall[:], in_=nxtall[:], axis=AX.X, op=OP.max
            )
            # eq -> (argC - BIG*eq) -> min = first argmax
            nc.vector.tensor_tensor(
                out=nxtall[:], in0=nxtall[:],
                in1=mall[:].unsqueeze(2).to_broadcast([PBJ, NT1, T]),
                op=OP.is_equal,
            )
            nc.vector.tensor_scalar_mul(
                out=nxtall[:], in0=nxtall[:], scalar1=BIG
            )
            nc.vector.tensor_tensor(
                out=nxtall[:],
                in0=argC_sb[:].unsqueeze(1).to_broadcast([PBJ, NT1, T]),
                in1=nxtall[:], op=OP.subtract,
            )
            hist_sb = crfp.tile([PBJ, NT1], FP)
            nc.vector.tensor_reduce(
                out=hist_sb[:], in_=nxtall[:], axis=AX.X, op=OP.min
            )
            nc.sync.dma_start(out=io["histv"][:, 0:NT1], in_=hist_sb[:])
            nc.sync.dma_start(
                out=io["histv"][:, S - 1 : S], in_=score_hist[:, SLS - 1 : SLS]
            )

            # ---- logsumexp forward in exp domain ----
            NBANK = (SLS - 2) // RENORM_EVERY + 2
            bank = crfp.tile([1, NBANK, BL], FP)
            nc.vector.memset(bank[:], 1.0)
            p_cur = crfp.tile([T, BL], FP, tag="pcur")
            nc.scalar.activation(
                out=p_cur[:], in_=emisT[:, 0:BL], func=AF.Exp, bias=start_sb[:]
            )
            nbank = 0
            for u in range(1, SLS):
                q_ps = psC2.tile([T, BL], FP, tag="qrb")
                nc.tensor.matmul(
                    out=q_ps[:], lhsT=expT_sb[:], rhs=p_cur[:], start=True,
                    stop=True,
                )
                nc.vector.tensor_tensor(
                    out=p_cur[:], in0=q_ps[:],
                    in1=expE[:, u * BL : (u + 1) * BL], op=OP.mult,
                )
                if u % RENORM_EVERY == 0:
                    s_ps = psC2.tile([1, BL * 3], FP, tag="s", name="sps")[:, :BL]
                    nc.tensor.matmul(
                        out=s_ps[:], lhsT=ones20[:], rhs=p_cur[:], start=True,
                        stop=True,
                    )
                    nc.vector.tensor_copy(
                        out=bank[:, nbank, :], in_=s_ps[:]
                    )
                    rc = crf2p.tile([1, BL], FP, tag="rc")
                    nc.vector.reciprocal(out=rc[:], in_=s_ps[:])
                    rb_ps = psC2.tile([T, BL], FP, tag="qrb")
                    nc.tensor.matmul(
                        out=rb_ps[:], lhsT=ones_row[:, :T],
                        rhs=rc[:], start=True, stop=True,
                    )
                    nc.vector.tensor_tensor(
                        out=p_cur[:], in0=p_cur[:], in1=rb_ps[:], op=OP.mult
                    )
                    nbank += 1
            # end transitions
            nc.vector.tensor_tensor(
                out=p_cur[:], in0=p_cur[:],
                in1=expEnd_sb[:].to_broadcast([T, BL]), op=OP.mult,
            )
            s_ps = psC2.tile([1, BL * 3], FP, tag="s", name="sps")[:, :BL]
            nc.tensor.matmul(
                out=s_ps[:], lhsT=ones20[:], rhs=p_cur[:], start=True, stop=True
            )
            nc.vector.tensor_copy(out=bank[:, nbank, :], in_=s_ps[:])
            nbank += 1
            logb = crfp.tile([1, NBANK, BL], FP)
            nc.scalar.activation(out=logb[:], in_=bank[:], func=AF.Ln)
            den = crfp.tile([1, BL], FP)
            nc.vector.tensor_reduce(
                out=den[:],
                in_=logb[:].rearrange("a n b -> a b n"),
                axis=AX.X,
                op=OP.add,
            )
            if debug:
                nc.sync.dma_start(out=io["dbg_den"][:, :], in_=den[:])
                nc.sync.dma_start(out=io["dbg_bank"][:, :, :], in_=bank[:])

            # ---- numerator ----
            # trans term: sum_t trans[tag_t, tag_{t+1}]
            npairs = (SLS - 1) * BL
            tsum = crfp.tile([T, SLS - 1, BL], FP)
            for q in range((npairs + 499) // 500):
                ps = psC.tile([PBJ, 500], FP, tag="big", name="ntr")[:T, :]
                c0 = q * 500
                cn = min(500, npairs - c0)
                nc.tensor.matmul(
                    out=ps[:, :cn], lhsT=transT_sb[:],
                    rhs=ohtags[:, BL + c0 : BL + c0 + cn], start=True, stop=True,
                )
                nc.vector.tensor_tensor(
                    out=tsum[:].rearrange("p a b -> p (a b)")[:, c0 : c0 + cn],
                    in0=ps[:, :cn],
                    in1=ohtags[:, c0 : c0 + cn],
                    op=OP.mult,
                )
            # emission term: sum_t e_t[b, tag_t]
            esum = crfp.tile([T, SLS, BL], FP)
            nc.vector.tensor_tensor(
                out=esum[:].rearrange("p a b -> p (a b)"),
                in0=emisT[:, : SLS * BL],
                in1=ohtags[:, : SLS * BL],
                op=OP.mult,
            )
            # start/end terms
            se = crfp.tile([T, 2, BL], FP)
            nc.vector.tensor_tensor(
                out=se[:, 0, :], in0=ohtags[:, 0:BL],
                in1=start_sb[:].to_broadcast([T, BL]), op=OP.mult,
            )
            nc.vector.tensor_tensor(
                out=se[:, 1, :], in0=ohtags[:, (SLS - 1) * BL : SLS * BL],
                in1=end_sb[:].to_broadcast([T, BL]), op=OP.mult,
            )
            # reduce over time (strided views -> [T, BL]), then over tags via
            # accumulating ones-matmuls into one psum
            tred = crfp.tile([T, 3, BL], FP)
            nc.vector.tensor_reduce(
                out=tred[:, 0, :],
                in_=tsum[:].rearrange("p a b -> p b a"),
                axis=AX.X, op=OP.add,
            )
            nc.vector.tensor_reduce(
                out=tred[:, 1, :],
                in_=esum[:].rearrange("p a b -> p b a"),
                axis=AX.X, op=OP.add,
            )
            nc.vector.tensor_reduce(
                out=tred[:, 2, :],
                in_=se[:].rearrange("p a b -> p b a"),
                axis=AX.X, op=OP.add,
            )
            num_ps = psC2.tile([1, BL * 3], FP, tag="s", name="nps")[:, : 3 * BL]
            nc.tensor.matmul(
                out=num_ps[:],
                lhsT=ones20[:],
                rhs=tred[:].rearrange("p a b -> p (a b)"),
                start=True,
                stop=True,
            )
            numt = crfp.tile([1, 3, BL], FP)
            nc.vector.tensor_copy(
                out=numt[:].rearrange("p a b -> p (a b)"), in_=num_ps[:]
            )
            num = crfp.tile([1, BL], FP)
            nc.vector.tensor_reduce(
                out=num[:], in_=numt[:].rearrange("p a b -> p b a"),
                axis=AX.X, op=OP.add,
            )
            if debug:
                nc.sync.dma_start(out=io["dbg_num"][:, :], in_=num[:])
            l4 = crfp.tile([1, BL], FP)
            nc.vector.tensor_tensor(
                out=l4[:], in0=den[:], in1=num[:], op=OP.subtract
            )
            lsum = crfp.tile([1, 1], FP)
            nc.vector.tensor_reduce(out=lsum[:], in_=l4[:], axis=AX.X, op=OP.add)
            nc.sync.dma_start(out=io["loss_part"][:, :], in_=lsum[:])


# ---------------------------------------------------------------------------
# host wrapper
# ---------------------------------------------------------------------------

_CACHE = {}


def _get_nc(debug=False):
    key = bool(debug)
    if key not in _CACHE:
        _CACHE[key] = build_program(debug)
    return _CACHE[key]


def run_device(inputs, debug=False, trace=False):
    nc = _get_nc(debug)
    shared = prep_shared(inputs)
    in_maps = []
    for c in range(NC):
        m = dict(shared)
        m.update(prep_core(inputs, c))
        in_maps.append(m)
    res = run_bass_kernel_spmd(
        nc, in_maps, core_ids=list(range(NC)), trace=trace
    )
    return res


def assemble_outputs(res):
    decode = np.zeros((B, S), np.int32)
    attnw = np.zeros((B, S, S), np.float32)
    loss = np.float32(0.0)
    for c in range(NC):
        r = res.results[c]
        b0 = c * BL
        aw = r["attnw"]                        # [BL, kp, kh, q]
        attnw[b0 : b0 + BL] = np.ascontiguousarray(
            aw.transpose(0, 3, 2, 1).reshape(BL, S, S)
        )
        loss += r["loss_part"][0, 0]
        hv = r["histv"].reshape(T, BL, S)      # [(j,b), s]
        fin = hv[:, :, S - 1]                  # [T, BL] final score + end
        hist = np.rint(hv[:, :, : S - 1]).astype(np.int32)  # [T, BL, S-1]
        last = np.argmax(fin, axis=0)          # [BL]
        dec = np.zeros((BL, S), np.int32)
        dec[:, S - 1] = last
        cur = last
        for t in range(S - 2, -1, -1):
            cur = hist[cur, np.arange(BL), t]
            dec[:, t] = cur
        decode[b0 : b0 + BL] = dec
    return decode, loss, attnw


def kernel(**inputs):
    res = run_device(inputs, debug=False, trace=False)
    return assemble_outputs(res)
